# revision 1
# baseline (speedup 1.0000x reference)
"""Trainium2 Bass kernel for nn_Graph_Enhance_model (GNN message passing).

Self-contained: hardcodes shapes B=4,F=32,H=8,O=16,D=2048, 8 cores.
Data-parallel over the 128 (b,f) frames: 16 frames per core.
"""

import os
import sys

for _p in ("/opt/trn_rl_repo", "/opt/pypackages"):
    if _p not in sys.path and os.path.isdir(_p):
        sys.path.append(_p)

import numpy as np
import ml_dtypes

import concourse.bass as bass
import concourse.bacc as bacc
import concourse.tile as tile
import concourse.mybir as mybir
from concourse import bass_utils
from concourse.masks import make_identity

BF16 = mybir.dt.bfloat16
F32 = mybir.dt.float32
AF = mybir.ActivationFunctionType
ALU = mybir.AluOpType
AX = mybir.AxisListType

NB = ml_dtypes.bfloat16

B, F, H, O, D = 4, 32, 8, 16, 2048
NFRAMES = B * F          # 128
NCORES = 8
FPC = NFRAMES // NCORES  # 16 frames per core
ROWS = H * O             # 128 rows per frame
KC = D // 128            # 16 K-chunks
NQ = FPC // 4            # 4 quads of 4 frames

_CACHE = {}




def _combine_e(nc, step, mt, q, pe, wb_sb, bet_sb, um1t, msum_f, pool):
    """UM = (msg_e_psum + be) * w ; step 1 also reduces over o into msum."""
    if step == 0:
        nc.vector.scalar_tensor_tensor(out=um1t[:, mt, :], in0=pe,
                                       scalar=bet_sb[:, mt:mt + 1], in1=wb_sb,
                                       op0=ALU.add, op1=ALU.mult)
    else:
        tmp = pool.tile([128, 512], F32, tag="um2")
        nc.vector.scalar_tensor_tensor(out=tmp, in0=pe,
                                       scalar=bet_sb[:, mt:mt + 1], in1=wb_sb,
                                       op0=ALU.add, op1=ALU.mult)
        nc.vector.reduce_sum(msum_f[:, mt, q * 32:(q + 1) * 32],
                             tmp.rearrange("p (f h o) -> p f h o", f=4, h=8),
                             axis=AX.X)


def _build_nc():
    nc = bacc.Bacc("TRN2", target_bir_lowering=False, debug=False, num_devices=NCORES)

    dt_in = {}

    def din(name, shape, dt):
        dt_in[name] = nc.dram_tensor(name, shape, dt, kind="ExternalInput")
        return dt_in[name]

    e0t = din("e0t", [NQ, D, 512], BF16)
    ot = din("ot", [D, FPC * O], BF16)
    ht_bd = din("ht_b", [D, FPC * H], BF16)
    h_rmd = din("h_rm", [FPC * H, D], F32)
    pmatd = din("pmat", [FPC * H, FPC], BF16)
    scsf = din("scsf", [D, 3 * FPC], BF16)
    sc4rm = din("sc4rm", [FPC, D], F32)
    sfrm = din("sfrm", [FPC, D], F32)
    wcat = din("wcat", [D, D], BF16)
    bl1td = din("bl1t", [128, 8], F32)
    betd = din("bet", [128, 8], F32)
    wnt = din("wnt", [D, D // 2], BF16)
    wnb = din("wnb", [1, D // 2], BF16)
    wl2 = din("wl2", [128, 8], BF16)
    ghi = din("ghi", [D, 3 * D], BF16)
    ghib = din("ghib", [1, 3 * D], BF16)
    ghh = din("ghh", [D, 3 * D], BF16)
    ghhb = din("ghhb", [1, 3 * D], BF16)
    gsi = din("gsi", [D, 3 * D], BF16)
    gsib = din("gsib", [1, 3 * D], BF16)
    gsh = din("gsh", [D, 3 * D], BF16)
    gshb = din("gshb", [1, 3 * D], BF16)
    outp = nc.dram_tensor("outp", [FPC, D], F32, kind="ExternalOutput")

    from contextlib import ExitStack

    with tile.TileContext(nc) as tc, ExitStack() as ctx:
        glob = ctx.enter_context(tc.tile_pool(name="glob", bufs=1))

        ones_b = glob.tile([1, 512], BF16)
        nc.vector.memset(ones_b, 1.0)
        ones16 = glob.tile([1, 16], BF16)
        nc.vector.memset(ones16, 1.0)
        ident16 = glob.tile([16, 16], BF16)
        make_identity(nc, ident16)

        wl2_sb = glob.tile([128, 8], BF16)
        nc.sync.dma_start(out=wl2_sb, in_=wl2.ap())
        bl1t_sb = glob.tile([128, 8], F32)
        nc.sync.dma_start(out=bl1t_sb, in_=bl1td.ap())
        bet_sb = glob.tile([128, 8], F32)
        nc.sync.dma_start(out=bet_sb, in_=betd.ap())

        msgn_sb = glob.tile([128, 8, FPC * O], BF16)    # [1024, 256] transposed msg_n
        msum_f = glob.tile([128, KC, FPC * H], F32)     # M_sum2^T (raw sum over o)
        msum_b = glob.tile([128, KC, FPC * H], BF16)
        ah_pad = glob.tile([128, KC, 3 * FPC], BF16)    # [All_human^T/8 | zeros]
        nc.vector.memset(ah_pad, 0.0)
        ht_b = glob.tile([128, KC, FPC * H], BF16)
        scsf_b = glob.tile([128, KC, 3 * FPC], BF16)    # [S_C4^T | 0 | S_f^T]

        nc.sync.dma_start(out=ht_b, in_=ht_bd.ap().rearrange("(kc p) n -> p kc n", p=128))
        nc.sync.dma_start(out=scsf_b, in_=scsf.ap().rearrange("(kc p) n -> p kc n", p=128))

        with (
            tc.tile_pool(name="pwcat", bufs=1) as pwcat,
            tc.tile_pool(name="pa", bufs=2) as pa,
            tc.tile_pool(name="pa1", bufs=1) as pa1,
        ):
            wcat_sb = pwcat.tile([128, KC, D], BF16)
            nc.sync.dma_start(out=wcat_sb, in_=wcat.ap().rearrange("(kc p) m -> p kc m", p=128))

            # ---------------- Phase 0: msg_n^T = Wn @ O^T + bn ----------------
            with (
                tc.tile_pool(name="p0", bufs=1) as p0,
                tc.tile_pool(name="p0ps", bufs=4, space="PSUM") as p0ps,
            ):
                wnb_sb = p0.tile([1, D // 2], BF16)
                nc.sync.dma_start(out=wnb_sb, in_=wnb.ap())
                ot_sb = p0.tile([128, KC, FPC * O], BF16)
                nc.sync.dma_start(out=ot_sb, in_=ot.ap().rearrange("(kc p) n -> p kc n", p=128))
                for half in range(2):
                    wn_sb = p0.tile([128, KC, 512], BF16, tag="wn")
                    nc.sync.dma_start(out=wn_sb, in_=wnt.ap()[:, half * 512:(half + 1) * 512]
                                      .rearrange("(kc p) m -> p kc m", p=128))
                    for mt4 in range(4):
                        mt = half * 4 + mt4
                        pm = p0ps.tile([128, FPC * O], F32, tag="pm")
                        for kc in range(KC):
                            nc.tensor.matmul(pm, lhsT=wn_sb[:, kc, mt4 * 128:(mt4 + 1) * 128],
                                             rhs=ot_sb[:, kc, :], start=(kc == 0), stop=False)
                        nc.tensor.matmul(pm, lhsT=wnb_sb[0:1, mt * 128:(mt + 1) * 128],
                                         rhs=ones_b[0:1, 0:FPC * O], start=False, stop=True)
                        nc.scalar.copy(msgn_sb[:, mt, :], pm)

        # ---------------- Phase A: 2 propagation steps over edges ----------------
            with tc.tile_pool(name="paps", bufs=4, space="PSUM") as paps, \
                 tc.tile_pool(name="papss", bufs=2, space="PSUM") as papss:
                for q in range(NQ):
                    xq = pa.tile([128, KC, 512], BF16, tag="xq")
                    nc.sync.dma_start(out=xq, in_=e0t.ap()[q].rearrange("(kc p) n -> p kc n", p=128))
                    um1t = pa1.tile([128, KC, 512], BF16, tag="um1t")
                    for step in range(2):
                        rhs = xq if step == 0 else um1t
                        # --- a-wave: relu(X @ Wl1^T + bl1), transposed ---
                        relu_sb = pa1.tile([128, 8, 512], BF16, tag="relu")
                        for mt in range(8, 16):
                            pw_a = paps.tile([128, 512], F32, tag="wave")
                            for kc in range(KC):
                                nc.tensor.matmul(pw_a, lhsT=wcat_sb[:, kc, mt * 128:(mt + 1) * 128],
                                                 rhs=rhs[:, kc, :], start=(kc == 0), stop=(kc == KC - 1))
                            nc.scalar.activation(relu_sb[:, mt - 8, :], pw_a, AF.Relu,
                                                 bias=bl1t_sb[:, mt - 8:mt - 7])
                        # --- logits + softmax over o (groups of 16) ---
                        pl = papss.tile([1, 512], F32, tag="pl")
                        for kc2 in range(8):
                            nc.tensor.matmul(pl, lhsT=wl2_sb[:, kc2:kc2 + 1],
                                             rhs=relu_sb[:, kc2, :], start=(kc2 == 0), stop=(kc2 == 7))
                        pl3 = pl.rearrange("o (g i) -> o g i", i=16)
                        mx = pa1.tile([1, 32], F32, tag="mx")
                        nc.vector.reduce_max(mx, pl3, axis=AX.X)
                        sub = pa1.tile([1, 512], F32, tag="sub")
                        nc.vector.tensor_tensor(sub.rearrange("o (g i) -> o g i", i=16), pl3,
                                                mx.broadcast_to((1, 32, 16)), op=ALU.subtract)
                        nc.scalar.activation(sub, sub, AF.Exp)
                        ex3 = sub.rearrange("o (g i) -> o g i", i=16)
                        sm = pa1.tile([1, 32], F32, tag="sm")
                        nc.vector.reduce_sum(sm, ex3, axis=AX.X)
                        rs = pa1.tile([1, 32], F32, tag="rs")
                        nc.vector.reciprocal(rs, sm)
                        w_sb = pa1.tile([1, 512], BF16, tag="w")
                        nc.vector.tensor_tensor(w_sb.rearrange("o (g i) -> o g i", i=16), ex3,
                                                rs.broadcast_to((1, 32, 16)), op=ALU.mult)
                        # --- msg_e wave; w-broadcast MM emitted after 2 groups ---
                        e_ps = []
                        wb_sb = pa1.tile([128, 512], F32, tag="wb")
                        for mt in range(8):
                            pe = paps.tile([128, 512], F32, tag="wave")
                            for kc in range(KC):
                                nc.tensor.matmul(pe, lhsT=wcat_sb[:, kc, mt * 128:(mt + 1) * 128],
                                                 rhs=rhs[:, kc, :], start=(kc == 0), stop=(kc == KC - 1))
                            e_ps.append(pe)
                            if mt == 1:
                                # broadcast w along partitions via K=1 matmul (PE waits
                                # here on softmax, hidden under the first 2 MM groups)
                                pw_b = papss.tile([128, 512], F32, tag="pw")
                                nc.tensor.matmul(pw_b, lhsT=ones_b[0:1, 0:128], rhs=w_sb,
                                                 start=True, stop=True)
                                nc.scalar.copy(wb_sb, pw_b)
                            if mt >= 1:
                                for cmt in ([0, 1] if mt == 1 else [mt]):
                                    _combine_e(nc, step, cmt, q, e_ps[cmt], wb_sb, bet_sb,
                                               um1t, msum_f, pa1)
                        wb4 = wb_sb.rearrange("p (f h o) -> p f h o", f=4, h=8)
                        # msg_n half (tiles 8..16): broadcast over h
                        for j in range(8):
                            mt = 8 + j
                            base = msgn_sb[:, j, q * 64:(q + 1) * 64]
                            mn_bc = bass.AP(tensor=base.tensor, offset=base.offset,
                                            ap=[list(base.ap[0]), [16, 4], [0, 8], [1, 16]])
                            if step == 0:
                                nc.vector.tensor_tensor(
                                    um1t[:, mt, :].rearrange("p (f h o) -> p f h o", f=4, h=8),
                                    mn_bc, wb4, op=ALU.mult)
                            else:
                                tmp = pa1.tile([128, 512], F32, tag="um2")
                                nc.vector.tensor_tensor(
                                    tmp.rearrange("p (f h o) -> p f h o", f=4, h=8),
                                    mn_bc, wb4, op=ALU.mult)
                                nc.vector.reduce_sum(msum_f[:, mt, q * 32:(q + 1) * 32],
                                                     tmp.rearrange("p (f h o) -> p f h o", f=4, h=8),
                                                     axis=AX.X)


            for kc in range(KC):
                nc.vector.tensor_copy(msum_b[:, kc, :], msum_f[:, kc, :])

        # ---------------- Phase B: human GRU (row-major, weights moving) ----------------
        with (
            tc.tile_pool(name="pb", bufs=2) as pb,
            tc.tile_pool(name="pb1", bufs=1) as pb1,
            tc.tile_pool(name="pbps", bufs=1, space="PSUM") as pbps,
            tc.tile_pool(name="pbps2", bufs=2, space="PSUM") as pbps2,
        ):
            NR = FPC * H  # 128 rows
            h_rm = pb1.tile([NR, D], F32)
            nc.sync.dma_start(out=h_rm, in_=h_rmd.ap())
            pmat_sb = pb1.tile([NR, FPC], BF16)
            nc.sync.dma_start(out=pmat_sb, in_=pmatd.ap())
            hum_b = pb1.tile([NR, D], BF16)

            def gh_block(j, pt, use_i, use_h, lastfix=None):
                """accumulate gi (wih@msum) and/or gh (whh@ht) for gate block j
                into psum pt, row-major [128 rows, 512 gates]."""
                ops = []
                if use_h:
                    wb_t = pb.tile([128, KC, 512], BF16, tag="bwh")
                    nc.sync.dma_start(out=wb_t, in_=ghh.ap()[:, j * 512:(j + 1) * 512]
                                      .rearrange("(kc p) m -> p kc m", p=128))
                    bb = pb.tile([1, 512], BF16, tag="bbh")
                    nc.sync.dma_start(out=bb, in_=ghhb.ap()[:, j * 512:(j + 1) * 512])
                    ops += [(wb_t, ht_b, kc) for kc in range(KC)] + [(bb, None, None)]
                if use_i:
                    wi_t = pb.tile([128, KC, 512], BF16, tag="bwi")
                    nc.sync.dma_start(out=wi_t, in_=ghi.ap()[:, j * 512:(j + 1) * 512]
                                      .rearrange("(kc p) m -> p kc m", p=128))
                    bi = pb.tile([1, 512], BF16, tag="bbi")
                    nc.sync.dma_start(out=bi, in_=ghib.ap()[:, j * 512:(j + 1) * 512])
                    ops += [(wi_t, msum_b, kc) for kc in range(KC)] + [(bi, None, None)]
                for idx, (w, x, kc) in enumerate(ops):
                    st, sp = idx == 0, idx == len(ops) - 1
                    if x is None:
                        nc.tensor.matmul(pt, lhsT=ones_b[0:1, 0:128], rhs=w[0:1, :],
                                         start=st, stop=sp)
                    else:
                        nc.tensor.matmul(pt, lhsT=x[:, kc, :], rhs=w[:, kc, :],
                                         start=st, stop=sp)

            for t in range(4):
                cols = slice(t * 512, (t + 1) * 512)
                p_r = pbps.tile([NR, 512], F32, tag="pr")
                gh_block(t, p_r, True, True)
                p_z = pbps.tile([NR, 512], F32, tag="pz")
                gh_block(4 + t, p_z, True, True)
                p_in = pbps.tile([NR, 512], F32, tag="pin")
                gh_block(8 + t, p_in, True, False)
                p_hn = pbps.tile([NR, 512], F32, tag="phn")
                gh_block(8 + t, p_hn, False, True)
                r_sb = pb1.tile([NR, 512], F32, tag="r")
                nc.scalar.activation(r_sb, p_r, AF.Sigmoid)
                z_sb = pb1.tile([NR, 512], F32, tag="z")
                nc.scalar.activation(z_sb, p_z, AF.Sigmoid)
                t1 = pb1.tile([NR, 512], F32, tag="t1")
                nc.vector.tensor_tensor(t1, r_sb, p_hn, op=ALU.mult)
                t2 = pb1.tile([NR, 512], F32, tag="t2")
                nc.vector.tensor_tensor(t2, t1, p_in, op=ALU.add)
                n_sb = pb1.tile([NR, 512], F32, tag="n")
                nc.scalar.activation(n_sb, t2, AF.Tanh)
                t3 = pb1.tile([NR, 512], F32, tag="t3")
                nc.vector.tensor_tensor(t3, h_rm[:, cols], n_sb, op=ALU.subtract)
                t4 = pb1.tile([NR, 512], F32, tag="t4")
                nc.vector.tensor_tensor(t4, z_sb, t3, op=ALU.mult)
                nc.vector.tensor_tensor(hum_b[:, cols], n_sb, t4, op=ALU.add)
            # All_human^T chunks via PE: ah[c] = hum[:, c-chunk].T @ pmat
            for c in range(KC):
                pah = pbps2.tile([128, FPC], F32, tag="pah")
                nc.tensor.matmul(pah, lhsT=hum_b[:, c * 128:(c + 1) * 128], rhs=pmat_sb,
                                 start=True, stop=True)
                nc.scalar.copy(ah_pad[:, c, 0:FPC], pah)


        # ---------------- Phase C: two S-node GRUs (stacked M=32 stationaries) ----------------
        with (
            tc.tile_pool(name="pc", bufs=2) as pc,
            tc.tile_pool(name="pc1", bufs=1) as pc1,
            tc.tile_pool(name="pcsm", bufs=1) as pcsm,
            tc.tile_pool(name="pcps", bufs=2, space="PSUM") as pcps,
            tc.tile_pool(name="pctps", bufs=2, space="PSUM") as pctps,
        ):
            sc4rm_sb = pc1.tile([FPC, D], F32)
            nc.sync.dma_start(out=sc4rm_sb, in_=sc4rm.ap())
            sfrm32 = pc1.tile([3 * FPC, D], F32)
            nc.sync.dma_start(out=sfrm32[2 * FPC:3 * FPC, :], in_=sfrm.ap())
            g1_sb = pc1.tile([16, 12, 512], BF16)    # gi1+gh1 (r,z) / inn (n-blocks)
            gh1n_sb = pc1.tile([16, 4, 512], BF16)   # hn1
            gh2_sb = pc1.tile([48, 12, 512], BF16)   # whh @ sf + bhh (rows 32:48)
            g2i_sb = pc1.tile([48, 12, 512], BF16)
            s1_sb = pc1.tile([16, D], BF16)
            s1t_pad = pc1.tile([128, KC, 3 * FPC], BF16)   # [zeros | s1^T]
            nc.vector.memset(s1t_pad, 0.0)
            out32 = pc1.tile([3 * FPC, D], F32)

            for j in range(12):
                wsi = pc.tile([128, KC, 512], BF16, tag="wsi")
                nc.sync.dma_start(out=wsi, in_=gsi.ap()[:, j * 512:(j + 1) * 512].rearrange("(kc p) m -> p kc m", p=128))
                wsh = pc.tile([128, KC, 512], BF16, tag="wsh")
                nc.sync.dma_start(out=wsh, in_=gsh.ap()[:, j * 512:(j + 1) * 512].rearrange("(kc p) m -> p kc m", p=128))
                bsi = pc.tile([1, 512], BF16, tag="bsi")
                nc.sync.dma_start(out=bsi, in_=gsib.ap()[:, j * 512:(j + 1) * 512])
                bsh = pc.tile([1, 512], BF16, tag="bsh")
                nc.sync.dma_start(out=bsh, in_=gshb.ap()[:, j * 512:(j + 1) * 512])
                # PA rows 0:16 = gi1 (+bsi); rows 16:32 = zeros
                PA = pcps.tile([48, 512], F32, tag="PA")
                for kc in range(KC):
                    nc.tensor.matmul(PA, lhsT=ah_pad[:, kc, :], rhs=wsi[:, kc, :],
                                     start=(kc == 0), stop=False)
                nc.tensor.matmul(PA[0:16, :], lhsT=ones16, rhs=bsi[0:1, :], start=False, stop=(j >= 8))
                if j < 8:
                    # accumulate: rows 0:16 += gh1, rows 16:32 += gh2; +bsh on all
                    for kc in range(KC):
                        nc.tensor.matmul(PA, lhsT=scsf_b[:, kc, :], rhs=wsh[:, kc, :],
                                         start=False, stop=False)
                    nc.tensor.matmul(PA, lhsT=ones_b[0:1, 0:48], rhs=bsh[0:1, :],
                                     start=False, stop=True)
                    nc.scalar.copy(g1_sb[:, j, :], PA[0:16, :])
                    nc.scalar.copy(gh2_sb[32:48, j, :], PA[32:48, :])
                else:
                    nc.scalar.copy(g1_sb[:, j, :], PA[0:16, :])
                    PH = pcps.tile([48, 512], F32, tag="PH")
                    for kc in range(KC):
                        nc.tensor.matmul(PH, lhsT=scsf_b[:, kc, :], rhs=wsh[:, kc, :],
                                         start=(kc == 0), stop=False)
                    nc.tensor.matmul(PH, lhsT=ones_b[0:1, 0:48], rhs=bsh[0:1, :],
                                     start=False, stop=True)
                    nc.scalar.copy(gh1n_sb[:, j - 8, :], PH[0:16, :])
                    nc.scalar.copy(gh2_sb[32:48, j, :], PH[32:48, :])

            # step-1 elementwise -> s1 (rows 0:16)
            for t in range(4):
                cols = slice(t * 512, (t + 1) * 512)
                z1 = pcsm.tile([16, 512], F32, tag="z1")
                nc.scalar.activation(z1, g1_sb[:, 4 + t, :], AF.Sigmoid)
                r1 = pcsm.tile([16, 512], F32, tag="r1")
                nc.scalar.activation(r1, g1_sb[:, t, :], AF.Sigmoid)
                u1 = pcsm.tile([16, 512], F32, tag="u1")
                nc.vector.tensor_tensor(u1, r1, gh1n_sb[:, t, :], op=ALU.mult)
                u2 = pcsm.tile([16, 512], F32, tag="u2")
                nc.vector.tensor_tensor(u2, u1, g1_sb[:, 8 + t, :], op=ALU.add)
                n1 = pcsm.tile([16, 512], F32, tag="n1")
                nc.scalar.activation(n1, u2, AF.Tanh)
                u3 = pcsm.tile([16, 512], F32, tag="u3")
                nc.vector.tensor_tensor(u3, sc4rm_sb[:, cols], n1, op=ALU.subtract)
                u4 = pcsm.tile([16, 512], F32, tag="u4")
                nc.vector.tensor_tensor(u4, z1, u3, op=ALU.mult)
                nc.vector.tensor_tensor(s1_sb[:, cols], n1, u4, op=ALU.add)
            # transpose s1 -> s1t_pad cols 16:32 (bf16)
            for c in range(KC):
                ptp = pctps.tile([128, 16], BF16, tag="tp")
                nc.tensor.transpose(ptp, s1_sb[:, c * 128:(c + 1) * 128], ident16)
                nc.scalar.copy(s1t_pad[:, c, 2 * FPC:3 * FPC], ptp)
            # step 2: gi2 = wih @ s1 (+bih), rows 16:32
            for j in range(12):
                wsi = pc.tile([128, KC, 512], BF16, tag="wsi")
                nc.sync.dma_start(out=wsi, in_=gsi.ap()[:, j * 512:(j + 1) * 512].rearrange("(kc p) m -> p kc m", p=128))
                bsi = pc.tile([1, 512], BF16, tag="bsi")
                nc.sync.dma_start(out=bsi, in_=gsib.ap()[:, j * 512:(j + 1) * 512])
                PZ = pcps.tile([48, 512], F32, tag="PA")
                for kc in range(KC):
                    nc.tensor.matmul(PZ, lhsT=s1t_pad[:, kc, :], rhs=wsi[:, kc, :],
                                     start=(kc == 0), stop=False)
                nc.tensor.matmul(PZ, lhsT=ones_b[0:1, 0:48], rhs=bsi[0:1, :],
                                 start=False, stop=True)
                if j < 8:
                    nc.vector.tensor_tensor(g2i_sb[32:48, j, :], PZ[32:48, :],
                                            gh2_sb[32:48, j, :], op=ALU.add)
                else:
                    nc.scalar.copy(g2i_sb[32:48, j, :], PZ[32:48, :])
            # step-2 elementwise (rows 16:32) -> out
            for t in range(4):
                cols = slice(t * 512, (t + 1) * 512)
                z2 = pcsm.tile([48, 512], F32, tag="z2")
                nc.scalar.activation(z2[32:48, :], g2i_sb[32:48, 4 + t, :], AF.Sigmoid)
                r2 = pcsm.tile([48, 512], F32, tag="r2")
                nc.scalar.activation(r2[32:48, :], g2i_sb[32:48, t, :], AF.Sigmoid)
                v1 = pcsm.tile([48, 512], F32, tag="v1")
                nc.vector.tensor_tensor(v1[32:48, :], r2[32:48, :], gh2_sb[32:48, 8 + t, :], op=ALU.mult)
                v2 = pcsm.tile([48, 512], F32, tag="v2")
                nc.vector.tensor_tensor(v2[32:48, :], v1[32:48, :], g2i_sb[32:48, 8 + t, :], op=ALU.add)
                n2 = pcsm.tile([48, 512], F32, tag="n2")
                nc.scalar.activation(n2[32:48, :], v2[32:48, :], AF.Tanh)
                v3 = pcsm.tile([48, 512], F32, tag="v3")
                nc.vector.tensor_tensor(v3[32:48, :], sfrm32[32:48, cols], n2[32:48, :], op=ALU.subtract)
                v4 = pcsm.tile([48, 512], F32, tag="v4")
                nc.vector.tensor_tensor(v4[32:48, :], z2[32:48, :], v3[32:48, :], op=ALU.mult)
                nc.vector.tensor_tensor(out32[32:48, cols], n2[32:48, :], v4[32:48, :], op=ALU.add)
            nc.sync.dma_start(out=outp.ap(), in_=out32[32:48, :])

    nc.compile()
    return nc


def _prep_in_maps(inputs):
    E = np.ascontiguousarray(inputs["H_O_edges"].reshape(NFRAMES, ROWS, D))
    On = inputs["O_nodes"].reshape(NFRAMES, O, D)
    Hn = inputs["H_nodes"].reshape(NFRAMES, H, D)
    Sc4 = inputs["S_node_C4"].reshape(NFRAMES, D)
    Sf = np.ascontiguousarray(inputs["final_S_node"].transpose(0, 2, 1)).reshape(NFRAMES, D)

    shared = {
        "wcat": np.ascontiguousarray(
            np.concatenate([inputs["We"], inputs["Wl1"]], axis=0).T).astype(NB),
        "bl1t": np.ascontiguousarray(inputs["bl1"].reshape(8, 128).T).astype(np.float32),
        "bet": np.ascontiguousarray(inputs["be"].reshape(8, 128).T).astype(np.float32),
        "pmat": np.ascontiguousarray(np.kron(np.eye(FPC), np.ones((H, 1))) / H).astype(NB),
        "wnt": np.ascontiguousarray(inputs["Wn"].T).astype(NB),
        "wnb": inputs["bn"][None, :].astype(NB),
        "wl2": np.ascontiguousarray(inputs["Wl2"][0].reshape(8, 128).T).astype(NB),
        "ghi": np.ascontiguousarray((inputs["gh_wih"] / float(O)).T).astype(NB),
        "ghib": inputs["gh_bih"][None, :].astype(NB),
        "ghh": np.ascontiguousarray(inputs["gh_whh"].T).astype(NB),
        "ghhb": inputs["gh_bhh"][None, :].astype(NB),
        "gsi": np.ascontiguousarray(inputs["gs_wih"].T).astype(NB),
        "gsib": inputs["gs_bih"][None, :].astype(NB),
        "gsh": np.ascontiguousarray(inputs["gs_whh"].T).astype(NB),
        "gshb": inputs["gs_bhh"][None, :].astype(NB),
    }

    in_maps = []
    for c in range(NCORES):
        fr = slice(c * FPC, (c + 1) * FPC)
        Ec = E[fr]  # [16, 128, 2048]
        e0t = np.ascontiguousarray(
            Ec.reshape(NQ, 4, ROWS, D).transpose(0, 3, 1, 2).reshape(NQ, D, 512)).astype(NB)
        ot = np.ascontiguousarray(
            On[fr].reshape(FPC * O, D).T).astype(NB)
        ht = np.ascontiguousarray(Hn[fr].reshape(FPC * H, D).T)
        sc4 = Sc4[fr]
        sf = Sf[fr]
        m = dict(shared)
        m.update({
            "e0t": e0t,
            "ot": ot,
            "ht_b": ht.astype(NB),
            "h_rm": np.ascontiguousarray(Hn[fr].reshape(FPC * H, D)).astype(np.float32),
            "scsf": np.ascontiguousarray(np.concatenate(
                [sc4.T, np.zeros_like(sc4.T), sf.T], axis=1)).astype(NB),
            "sc4rm": np.ascontiguousarray(sc4).astype(np.float32),
            "sfrm": np.ascontiguousarray(sf).astype(np.float32),
        })
        in_maps.append(m)
    return in_maps


LAST_RESULT = None


def kernel(**inputs):
    global LAST_RESULT
    if "nc" not in _CACHE:
        _CACHE["nc"] = _build_nc()
    nc = _CACHE["nc"]
    in_maps = _prep_in_maps(inputs)
    trace = os.environ.get("KERNEL_TRACE", "0") == "1"
    res = bass_utils.run_bass_kernel_spmd(
        nc, in_maps, core_ids=list(range(NCORES)), trace=trace)
    LAST_RESULT = res
    out = np.concatenate([res.results[c]["outp"] for c in range(NCORES)], axis=0)
    return np.ascontiguousarray(out.reshape(B, F, D)).astype(np.float32)


if __name__ == "__main__":
    np.random.seed(0)
    ins = {
        "S_node_C4": np.random.randn(B, F, D).astype(np.float32),
        "final_S_node": np.random.randn(B, D, F).astype(np.float32),
        "H_nodes": np.random.randn(B, F, H, D).astype(np.float32),
        "O_nodes": np.random.randn(B, F, O, D).astype(np.float32),
        "H_O_edges": np.random.randn(B, F, H, O, D).astype(np.float32),
        "Wn": np.random.randn(D // 2, D).astype(np.float32) * 0.02,
        "bn": np.random.randn(D // 2).astype(np.float32) * 0.02,
        "We": np.random.randn(D // 2, D).astype(np.float32) * 0.02,
        "be": np.random.randn(D // 2).astype(np.float32) * 0.02,
        "Wl1": np.random.randn(D // 2, D).astype(np.float32) * 0.02,
        "bl1": np.random.randn(D // 2).astype(np.float32) * 0.02,
        "Wl2": np.random.randn(1, D // 2).astype(np.float32) * 0.02,
        "bl2": np.random.randn(1).astype(np.float32) * 0.02,
        "gh_wih": np.random.randn(3 * D, D).astype(np.float32) * 0.02,
        "gh_whh": np.random.randn(3 * D, D).astype(np.float32) * 0.02,
        "gh_bih": np.random.randn(3 * D).astype(np.float32) * 0.02,
        "gh_bhh": np.random.randn(3 * D).astype(np.float32) * 0.02,
        "gs_wih": np.random.randn(3 * D, D).astype(np.float32) * 0.02,
        "gs_whh": np.random.randn(3 * D, D).astype(np.float32) * 0.02,
        "gs_bih": np.random.randn(3 * D).astype(np.float32) * 0.02,
        "gs_bhh": np.random.randn(3 * D).astype(np.float32) * 0.02,
    }
    out = kernel(**ins)
    print("kernel ran, out shape", out.shape, out.dtype, float(np.abs(out).mean()))



# revision 6
# speedup vs baseline: 1.2418x; 1.2418x over previous
"""Trainium2 Bass kernel for nn_Graph_Enhance_model (GNN message passing).

Self-contained: hardcodes shapes B=4,F=32,H=8,O=16,D=2048, 8 cores.
Data-parallel over the 128 (b,f) frames: 16 frames per core.

v2: algebraic step-1 restructure (E1 = w0*[me0;mn] is rank-structured, so
step-1 waves collapse to half-K matmuls on me0 plus per-o weighted
reductions folded before the We matmul), fp8e4m3 DoubleRow for the big
matmuls, fp8 weights for the human GRU, e3m4 wih / bf16 whh for the
S-node GRUs, col-group-packed small-M matmuls in the S-GRU phase.
"""

import os
import sys

for _p in ("/opt/trn_rl_repo", "/opt/pypackages"):
    if _p not in sys.path and os.path.isdir(_p):
        sys.path.append(_p)

import numpy as np
import ml_dtypes

import concourse.bass as bass
import concourse.bacc as bacc
import concourse.tile as tile
import concourse.mybir as mybir
from concourse import bass_utils
from concourse.masks import make_identity

BF16 = mybir.dt.bfloat16
F32 = mybir.dt.float32
F8 = mybir.dt.float8e4
F8E3 = mybir.dt.float8e3
AF = mybir.ActivationFunctionType
ALU = mybir.AluOpType
AX = mybir.AxisListType
DR = mybir.MatmulPerfMode.DoubleRow

NB = ml_dtypes.bfloat16
NE4 = ml_dtypes.float8_e4m3
NE3 = ml_dtypes.float8_e3m4

B, F, H, O, D = 4, 32, 8, 16, 2048
NFRAMES = B * F          # 128
NCORES = 8
FPC = NFRAMES // NCORES  # 16 frames per core
ROWS = H * O             # 128 rows per frame
KC = D // 128            # 16 K-chunks
NQ = FPC // 4            # 4 quads of 4 frames

WS = 8.0                 # fp8e4 weight scale
WS3 = 64.0               # e3m4 weight scale
GSH_E3 = False           # S-GRU whh in e3m4 (True) or bf16 (False)

_CACHE = {}


def _bc4(t, kc, q):
    """Broadcast-over-h AP: [128, 4f, 8h(stride0), 16o] of t[:, kc, q*64:(q+1)*64]."""
    base = t[:, kc, q * 64:(q + 1) * 64]
    return bass.AP(tensor=base.tensor, offset=base.offset,
                   ap=[list(base.ap[0]), [16, 4], [0, 8], [1, 16]])


def _r4(t):
    """[128, 512] -> [128, 4f, 8h, 16o]."""
    return t.rearrange("p (f h o) -> p f h o", f=4, h=8)


def _build_nc():
    nc = bacc.Bacc("TRN2", target_bir_lowering=False, debug=False, num_devices=NCORES)

    def din(name, shape, dt):
        return nc.dram_tensor(name, shape, dt, kind="ExternalInput")

    e0t = din("e0t", [NQ, D, 512], F8)
    ot = din("ot", [D, FPC * O], F8)
    wnt = din("wnt", [D, D // 2], F8)
    wcat = din("wcat", [D, D], F8)
    wl1l = din("wl1l", [D // 2, D // 2], F8)
    wl1r = din("wl1r", [D // 2, D // 2], F8)
    wl2 = din("wl2", [128, 8], BF16)
    bl1td = din("bl1t", [128, 8], BF16)
    bet8d = din("bet8", [1, D // 2], BF16)
    bnt8d = din("bnt8", [1, D // 2], BF16)
    ht8d = din("ht8", [D, FPC * H], F8)
    h_rmd = din("h_rm", [FPC * H, D], F32)
    pmatd = din("pmat", [FPC * H, FPC], BF16)
    ghi = din("ghi", [D, 3 * D], F8)
    ghh = din("ghh", [D, 3 * D], F8)
    ghibd = din("ghib", [1, 3 * D], BF16)
    ghhbd = din("ghhb", [1, 3 * D], BF16)
    gsid = din("gsi", [D, 3 * D], F8E3)
    gshd = din("gsh", [D, 3 * D], F8E3 if GSH_E3 else BF16)
    gsibd = din("gsib", [1, 3 * D], BF16)
    gshbd = din("gshb", [1, 3 * D], BF16)
    scsfd = din("scsf", [D, 2 * FPC], BF16)
    sc4rmd = din("sc4rm", [FPC, D], F32)
    sfrmd = din("sfrm", [FPC, D], F32)
    outp = nc.dram_tensor("outp", [FPC, D], F32, kind="ExternalOutput")

    SH = 1.0 / WS3 if GSH_E3 else 1.0   # descale for gsh-side psums
    GSH_DT = F8E3 if GSH_E3 else BF16

    from contextlib import ExitStack

    with tile.TileContext(nc) as tc, ExitStack() as ctx:
        glob = ctx.enter_context(tc.tile_pool(name="glob", bufs=1))
        pbias = ctx.enter_context(tc.tile_pool(name="pbias", bufs=3, side="right"))

        oi_t = glob.tile([16, 544], BF16)
        ident16 = oi_t[0:16, 0:16]
        make_identity(nc, ident16)
        ones_b = oi_t[0:1, 32:544]
        nc.vector.memset(ones_b, 1.0)
        wb_t = glob.tile([128, 16], BF16)
        wl2_sb = wb_t[:, 0:8]
        nc.sync.dma_start(out=wl2_sb, in_=wl2.ap())
        bl1t_sb = wb_t[:, 8:16]
        nc.sync.dma_start(out=bl1t_sb, in_=bl1td.ap())
        bb8_t = glob.tile([1, 2, D // 2], BF16)
        bet8_sb = bb8_t[:, 0, :]
        nc.sync.dma_start(out=bet8_sb, in_=bet8d.ap())
        bnt8_sb = bb8_t[:, 1, :]
        nc.sync.dma_start(out=bnt8_sb, in_=bnt8d.ap())

        f8pair = glob.tile([128, KC, 2 * FPC * H], F8)
        msum_f8 = f8pair[:, :, 0:FPC * H]
        ht8_sb = f8pair[:, :, FPC * H:2 * FPC * H]
        nc.sync.dma_start(out=ht8_sb, in_=ht8d.ap().rearrange("(kc p) n -> p kc n", p=128))
        bfpack = glob.tile([128, KC, 4 * FPC], BF16)
        scsf_sb = bfpack[:, :, 0:2 * FPC]
        nc.sync.dma_start(out=scsf_sb, in_=scsfd.ap().rearrange("(kc p) n -> p kc n", p=128))
        ah_sb = bfpack[:, :, 2 * FPC:3 * FPC]
        s1t_sb = bfpack[:, :, 3 * FPC:4 * FPC]

        bw_tiles = {}
        bw_order = []
        for t in range(4):
            bw_order += [("h", t), ("i", t), ("h", 4 + t), ("i", 4 + t),
                         ("i", 8 + t), ("h", 8 + t)]

        with tc.tile_pool(name="bw", bufs=4, side="right") as bwpool:

            def bw_load(src, j):
                wt = bwpool.tile([128, KC, 512], F8, tag="bw")
                mat = ghh if src == "h" else ghi
                nc.sync.dma_start(out=wt, in_=mat.ap()[:, j * 512:(j + 1) * 512]
                                  .rearrange("(kc p) m -> p kc m", p=128))
                bw_tiles[(src, j)] = wt

            # ================= Phase A =================
            with (
                tc.tile_pool(name="pal", bufs=1) as pal,
                tc.tile_pool(name="pwcat", bufs=1) as pwcat,
                tc.tile_pool(name="pa", bufs=2) as pa,
                tc.tile_pool(name="pam", bufs=1) as pam,
                tc.tile_pool(name="pa1", bufs=1) as pa1,
                tc.tile_pool(name="pav", bufs=2) as pav,
            ):
                mn_f8 = pal.tile([128, 8, FPC * O], F8)       # mn^T, unscaled
                q8_sb = pal.tile([128, 8, FPC * O], BF16)     # 8 * (Wl1R mn)
                xu_f = pal.tile([128, KC, FPC * H], F32)      # (me0u ⊕ mnu)^T
                xu_b = pal.tile([128, KC, FPC * H], BF16)
                msum_f = pal.tile([128, KC, FPC * H], F32)    # msum^T (raw sum over o)

                wcat_sb = pwcat.tile([128, KC, D], F8)
                nc.sync.dma_start(out=wcat_sb, in_=wcat.ap().rearrange("(kc p) m -> p kc m", p=128))
                wl1l_sb = pwcat.tile([128, 8, D // 2], F8)
                nc.sync.dma_start(out=wl1l_sb, in_=wl1l.ap().rearrange("(kc p) m -> p kc m", p=128))

                # ---- Phase 0: mn^T = Wn O^T + bn; Q = Wl1R mn ----
                with (
                    tc.tile_pool(name="p0", bufs=1) as p0,
                    tc.tile_pool(name="p0ps", bufs=4, space="PSUM") as p0ps,
                ):
                    wnt_sb = p0.tile([128, KC, D // 2], F8)
                    nc.sync.dma_start(out=wnt_sb, in_=wnt.ap().rearrange("(kc p) m -> p kc m", p=128))
                    ot_sb = p0.tile([128, KC, FPC * O], F8)
                    nc.sync.dma_start(out=ot_sb, in_=ot.ap().rearrange("(kc p) n -> p kc n", p=128))
                    wl1r_sb = p0.tile([128, 8, D // 2], F8)
                    nc.sync.dma_start(out=wl1r_sb, in_=wl1r.ap().rearrange("(kc p) m -> p kc m", p=128))
                    for mt in range(8):
                        pm = p0ps.tile([128, FPC * O], F32, tag="pm")
                        for i in range(8):
                            nc.tensor.matmul(pm, lhsT=wnt_sb[:, 2 * i:2 * i + 2, mt * 128:(mt + 1) * 128],
                                             rhs=ot_sb[:, 2 * i:2 * i + 2, :],
                                             perf_mode=DR, start=(i == 0), stop=False)
                        nc.tensor.matmul(pm, lhsT=bnt8_sb[0:1, mt * 128:(mt + 1) * 128],
                                         rhs=ones_b[0:1, 0:FPC * O], start=False, stop=True)
                        nc.scalar.activation(mn_f8[:, mt, :], pm, AF.Copy, scale=1.0 / WS)
                    for mt in range(8):
                        pq = p0ps.tile([128, FPC * O], F32, tag="pm")
                        for i in range(4):
                            nc.tensor.matmul(pq, lhsT=wl1r_sb[:, 2 * i:2 * i + 2, mt * 128:(mt + 1) * 128],
                                             rhs=mn_f8[:, 2 * i:2 * i + 2, :],
                                             perf_mode=DR, start=(i == 0), stop=(i == 3))
                        nc.scalar.copy(q8_sb[:, mt, :], pq)

                with tc.tile_pool(name="paps", bufs=4, space="PSUM") as paps, \
                     tc.tile_pool(name="papss", bufs=2, space="PSUM") as papss:

                    def softmax_block(relu_t, wtag):
                        pl = papss.tile([1, 512], F32, tag="pl")
                        for kc2 in range(8):
                            nc.tensor.matmul(pl, lhsT=wl2_sb[:, kc2:kc2 + 1],
                                             rhs=relu_t[:, kc2, :], start=(kc2 == 0), stop=(kc2 == 7))
                        pl3 = pl.rearrange("o (g i) -> o g i", i=16)
                        smt = pa1.tile([1, 96], F32, tag="smt")
                        mx, sm, rs = smt[:, 0:32], smt[:, 32:64], smt[:, 64:96]
                        nc.vector.reduce_max(mx, pl3, axis=AX.X)
                        sub = pa1.tile([1, 512], F32, tag="sub")
                        nc.vector.tensor_tensor(sub.rearrange("o (g i) -> o g i", i=16), pl3,
                                                mx.broadcast_to((1, 32, 16)), op=ALU.subtract)
                        nc.scalar.activation(sub, sub, AF.Exp)
                        ex3 = sub.rearrange("o (g i) -> o g i", i=16)
                        nc.vector.reduce_sum(sm, ex3, axis=AX.X)
                        nc.vector.reciprocal(rs, sm)
                        w_sb = wrow[:, 0 if wtag == "w0" else 1, :]
                        nc.vector.tensor_tensor(w_sb.rearrange("o (g i) -> o g i", i=16), ex3,
                                                rs.broadcast_to((1, 32, 16)), op=ALU.mult)
                        return w_sb

                    def broadcast_w(w_sb, bidx):
                        pw = papss.tile([128, 512], F32, tag="pw")
                        nc.tensor.matmul(pw, lhsT=ones_b[0:1, 0:128], rhs=w_sb,
                                         start=True, stop=True)
                        wb = wbb[:, bidx, :]
                        nc.scalar.copy(wb, pw)
                        return wb

                    for q in range(NQ):
                        xq = pa.tile([128, KC, 512], F8, tag="xq")
                        nc.sync.dma_start(out=xq, in_=e0t.ap()[q].rearrange("(kc p) n -> p kc n", p=128))
                        me0t = pam.tile([128, 8, 512], F8, tag="me0t")
                        relu_sb = pa1.tile([128, 8, 512], BF16, tag="relu")
                        wrow = pa1.tile([1, 2, 512], BF16, tag="wrow")
                        wbb = pa1.tile([128, 3, 512], BF16, tag="wbb")

                        # step0: me0 = We E + be
                        for mt in range(8):
                            pe = paps.tile([128, 512], F32, tag="wave")
                            for i in range(8):
                                nc.tensor.matmul(pe, lhsT=wcat_sb[:, 2 * i:2 * i + 2, mt * 128:(mt + 1) * 128],
                                                 rhs=xq[:, 2 * i:2 * i + 2, :],
                                                 perf_mode=DR, start=(i == 0), stop=False)
                            nc.tensor.matmul(pe, lhsT=bet8_sb[0:1, mt * 128:(mt + 1) * 128],
                                             rhs=ones_b[0:1, 0:512], start=False, stop=True)
                            nc.scalar.activation(me0t[:, mt, :], pe, AF.Copy, scale=1.0 / WS)
                        # step0: a0 = relu(Wl1 E + bl1)
                        for mt in range(8, 16):
                            pe = paps.tile([128, 512], F32, tag="wave")
                            for i in range(8):
                                nc.tensor.matmul(pe, lhsT=wcat_sb[:, 2 * i:2 * i + 2, mt * 128:(mt + 1) * 128],
                                                 rhs=xq[:, 2 * i:2 * i + 2, :],
                                                 perf_mode=DR, start=(i == 0), stop=(i == 7))
                            nc.scalar.activation(relu_sb[:, mt - 8, :], pe, AF.Relu,
                                                 bias=bl1t_sb[:, mt - 8:mt - 7], scale=1.0 / WS)
                        w0_sb = softmax_block(relu_sb, "w0")
                        w0b = broadcast_w(w0_sb, 0)

                        # step1: a1 = relu(w0*(P+Q) + bl1), P = Wl1L me0
                        for mt in range(8):
                            pp = paps.tile([128, 512], F32, tag="wave")
                            for i in range(4):
                                nc.tensor.matmul(pp, lhsT=wl1l_sb[:, 2 * i:2 * i + 2, mt * 128:(mt + 1) * 128],
                                                 rhs=me0t[:, 2 * i:2 * i + 2, :],
                                                 perf_mode=DR, start=(i == 0), stop=(i == 3))
                            v1 = pav.tile([128, 512], BF16, tag="v")
                            nc.vector.tensor_tensor(_r4(v1), _r4(pp), _bc4(q8_sb, mt, q), op=ALU.add)
                            v2 = pav.tile([128, 512], BF16, tag="v")
                            nc.vector.tensor_tensor(v2, v1, w0b, op=ALU.mult)
                            nc.scalar.activation(relu_sb[:, mt, :], v2, AF.Relu,
                                                 bias=bl1t_sb[:, mt:mt + 1], scale=1.0 / WS)
                        w1_sb = softmax_block(relu_sb, "w1")
                        w1b = broadcast_w(w1_sb, 1)
                        ub = wbb[:, 2, :]
                        nc.vector.tensor_tensor(ub, w0b, w1b, op=ALU.mult)

                        # weighted reductions over o
                        for kc in range(8):
                            tmp = pav.tile([128, 512], BF16, tag="tmp")
                            nc.vector.tensor_tensor(tmp, me0t[:, kc, :], ub, op=ALU.mult)
                            nc.vector.reduce_sum(xu_f[:, kc, q * 32:(q + 1) * 32], _r4(tmp), axis=AX.X)
                        for kc in range(8):
                            tmp = pav.tile([128, 512], BF16, tag="tmp")
                            nc.vector.tensor_tensor(_r4(tmp), _bc4(mn_f8, kc, q), _r4(ub), op=ALU.mult)
                            nc.vector.reduce_sum(xu_f[:, 8 + kc, q * 32:(q + 1) * 32], _r4(tmp), axis=AX.X)
                            tmp2 = pav.tile([128, 512], BF16, tag="tmp")
                            nc.vector.tensor_tensor(_r4(tmp2), _bc4(mn_f8, kc, q), _r4(w1b), op=ALU.mult)
                            nc.vector.reduce_sum(msum_f[:, 8 + kc, q * 32:(q + 1) * 32], _r4(tmp2), axis=AX.X)

                        if q == 0:  # prefetch first GRU weight blocks under phase A
                            for src, j in bw_order[:4]:
                                bw_load(src, j)

                    for kc in range(KC):
                        nc.vector.tensor_copy(xu_b[:, kc, :], xu_f[:, kc, :])
                    # folded msum_e = We (me0u ⊕ mnu) + be
                    for mt in range(8):
                        pf = papss.tile([128, FPC * H], F32, tag="pw")
                        for kc in range(KC):
                            nc.tensor.matmul(pf, lhsT=wcat_sb[:, kc, mt * 128:(mt + 1) * 128],
                                             rhs=xu_b[:, kc, :], start=(kc == 0), stop=False)
                        nc.tensor.matmul(pf, lhsT=bet8_sb[0:1, mt * 128:(mt + 1) * 128],
                                         rhs=ones_b[0:1, 0:FPC * H], start=False, stop=True)
                        nc.scalar.activation(msum_f[:, mt, :], pf, AF.Copy, scale=1.0 / WS)
                    for kc in range(KC):
                        nc.vector.tensor_copy(msum_f8[:, kc, :], msum_f[:, kc, :])

            # ================= Phase B: human GRU =================
            with (
                tc.tile_pool(name="pbh", bufs=1) as pbh,
                tc.tile_pool(name="pbt", bufs=2) as pbt,
                tc.tile_pool(name="pbps", bufs=1, space="PSUM") as pbps,
                tc.tile_pool(name="pbps2", bufs=2, space="PSUM") as pbps2,
            ):
                NR = FPC * H  # 128 rows
                h_rm = pbh.tile([NR, D], F32)
                nc.sync.dma_start(out=h_rm, in_=h_rmd.ap())
                pmat_sb = pbh.tile([NR, FPC], BF16)
                nc.sync.dma_start(out=pmat_sb, in_=pmatd.ap())
                hum_b = pbh.tile([NR, D], BF16)

                def gh_block(j, pt, use_i, use_h):
                    ops = []
                    if use_h:
                        ops += [("h", j, i) for i in range(8)] + [(ghhbd, None, None)]
                    if use_i:
                        ops += [("i", j, i) for i in range(8)] + [(ghibd, None, None)]
                    for idx, (src, jj, i) in enumerate(ops):
                        st, sp = idx == 0, idx == len(ops) - 1
                        if jj is None:
                            bb = pbias.tile([1, 512], BF16, tag="bias")
                            nc.sync.dma_start(out=bb, in_=src.ap()[:, j * 512:(j + 1) * 512])
                            nc.tensor.matmul(pt, lhsT=ones_b[0:1, 0:NR], rhs=bb,
                                             start=st, stop=sp)
                        else:
                            if (src, jj) not in bw_tiles:
                                bw_load(src, jj)
                            wt = bw_tiles[(src, jj)]
                            x = ht8_sb if src == "h" else msum_f8
                            nc.tensor.matmul(pt, lhsT=x[:, 2 * i:2 * i + 2, :],
                                             rhs=wt[:, 2 * i:2 * i + 2, :],
                                             perf_mode=DR, start=st, stop=sp)

                for t in range(4):
                    cols = slice(t * 512, (t + 1) * 512)
                    p_r = pbps.tile([NR, 512], F32, tag="pr")
                    gh_block(t, p_r, True, True)
                    p_z = pbps.tile([NR, 512], F32, tag="pz")
                    gh_block(4 + t, p_z, True, True)
                    p_in = pbps.tile([NR, 512], F32, tag="pin")
                    gh_block(8 + t, p_in, True, False)
                    p_hn = pbps.tile([NR, 512], F32, tag="phn")
                    gh_block(8 + t, p_hn, False, True)
                    r_sb = pbh.tile([NR, 512], F32, tag="r")
                    nc.scalar.activation(r_sb, p_r, AF.Sigmoid, scale=1.0 / WS)
                    z_sb = pbh.tile([NR, 512], F32, tag="z")
                    nc.scalar.activation(z_sb, p_z, AF.Sigmoid, scale=1.0 / WS)
                    t1 = pbt.tile([NR, 512], F32, tag="tt")
                    nc.vector.tensor_tensor(t1, r_sb, p_hn, op=ALU.mult)
                    t2 = pbt.tile([NR, 512], F32, tag="tt")
                    nc.vector.tensor_tensor(t2, t1, p_in, op=ALU.add)
                    n_sb = pbh.tile([NR, 512], F32, tag="n")
                    nc.scalar.activation(n_sb, t2, AF.Tanh, scale=1.0 / WS)
                    t3 = pbt.tile([NR, 512], F32, tag="tt")
                    nc.vector.tensor_tensor(t3, h_rm[:, cols], n_sb, op=ALU.subtract)
                    t4 = pbt.tile([NR, 512], F32, tag="tt")
                    nc.vector.tensor_tensor(t4, z_sb, t3, op=ALU.mult)
                    nc.vector.tensor_tensor(hum_b[:, cols], n_sb, t4, op=ALU.add)
                for c in range(KC):
                    pah = pbps2.tile([128, FPC], F32, tag="pah")
                    nc.tensor.matmul(pah, lhsT=hum_b[:, c * 128:(c + 1) * 128], rhs=pmat_sb,
                                     start=True, stop=True)
                    nc.scalar.copy(ah_sb[:, c, :], pah)

        # ================= Phase C: two S-node GRUs =================
        with (
            tc.tile_pool(name="pc1", bufs=1) as pc1,
            tc.tile_pool(name="pct", bufs=2) as pct,
            tc.tile_pool(name="pcw", bufs=3, side="right") as pcw,
            tc.tile_pool(name="pci", bufs=3, side="right") as pci,
            tc.tile_pool(name="pcps", bufs=2, space="PSUM") as pcps,
            tc.tile_pool(name="pctps", bufs=2, space="PSUM") as pctps,
        ):
            sc4rm_sb = pc1.tile([FPC, D], F32)
            nc.sync.dma_start(out=sc4rm_sb, in_=sc4rmd.ap())
            sfrm_sb = pc1.tile([FPC, D], F32)
            nc.sync.dma_start(out=sfrm_sb, in_=sfrmd.ap())
            gh1_sb = pc1.tile([FPC, 12, 512], BF16)   # whh Sc4 + bhh (unscaled)
            gh2_sb = pc1.tile([FPC, 12, 512], BF16)   # whh Sf + bhh
            g_sb = pc1.tile([FPC, 8, 512], BF16, tag="g")     # r,z gates (gi/WS3+gh)
            gn_sb = pc1.tile([FPC, 4, 512], BF16, tag="gn")   # inn
            s1_sb = pc1.tile([FPC, D], BF16)
            out32 = pc1.tile([FPC, D], F32)

            # PH pass: gh1 = whh Sc4 + bhh, gh2 = whh Sf + bhh; 2 j-blocks per psum
            for jp in range(6):
                pch1 = pcps.tile([128, 512], F32, tag="pch")
                pch2 = pcps.tile([128, 512], F32, tag="pch2")
                wts = []
                for g in range(2):
                    j = jp * 2 + g
                    wt = pcw.tile([128, KC, 512], GSH_DT, tag="cw")
                    nc.sync.dma_start(out=wt, in_=gshd.ap()[:, j * 512:(j + 1) * 512]
                                      .rearrange("(kc p) m -> p kc m", p=128))
                    wts.append(wt)
                for kc in range(KC):
                    for g in range(2):
                        nc.tensor.matmul(pch1[32 * g:32 * g + 16, :],
                                         lhsT=scsf_sb[:, kc, 0:FPC], rhs=wts[g][:, kc, :],
                                         tile_position=(0, 32 * g),
                                         start=(kc == 0), stop=False, skip_group_check=True)
                        nc.tensor.matmul(pch2[32 * g:32 * g + 16, :],
                                         lhsT=scsf_sb[:, kc, FPC:2 * FPC], rhs=wts[g][:, kc, :],
                                         tile_position=(0, 32 * g),
                                         start=(kc == 0), stop=False, skip_group_check=True)
                for g in range(2):
                    j = jp * 2 + g
                    bsh = pbias.tile([1, 512], BF16, tag="bias")
                    nc.sync.dma_start(out=bsh, in_=gshbd.ap()[:, j * 512:(j + 1) * 512])
                    nc.tensor.matmul(pch1[32 * g:32 * g + 16, :], lhsT=ones_b[0:1, 0:16],
                                     rhs=bsh, tile_position=(0, 32 * g),
                                     start=False, stop=True, skip_group_check=True)
                    nc.tensor.matmul(pch2[32 * g:32 * g + 16, :], lhsT=ones_b[0:1, 0:16],
                                     rhs=bsh, tile_position=(0, 32 * g),
                                     start=False, stop=True, skip_group_check=True)
                for g in range(2):
                    j = jp * 2 + g
                    nc.scalar.activation(gh1_sb[:, j, :], pch1[32 * g:32 * g + 16, :],
                                         AF.Copy, scale=SH)
                    nc.scalar.activation(gh2_sb[:, j, :], pch2[32 * g:32 * g + 16, :],
                                         AF.Copy, scale=SH)

            def gi_pass(xt, gh_src):
                """gi = wih x + bih (x64); g = gi/64 + gh for r,z; raw gi/64 for n."""
                for jp in range(6):
                    pci_ps = pcps.tile([128, 512], F32, tag="pch")
                    wts = []
                    for g in range(2):
                        j = jp * 2 + g
                        wt = pci.tile([128, KC, 512], F8E3, tag="ci")
                        nc.sync.dma_start(out=wt, in_=gsid.ap()[:, j * 512:(j + 1) * 512]
                                          .rearrange("(kc p) m -> p kc m", p=128))
                        wts.append(wt)
                    for kc in range(KC):
                        for g in range(2):
                            nc.tensor.matmul(pci_ps[32 * g:32 * g + 16, :],
                                             lhsT=xt[:, kc, :], rhs=wts[g][:, kc, :],
                                             tile_position=(0, 32 * g),
                                             start=(kc == 0), stop=False, skip_group_check=True)
                    for g in range(2):
                        j = jp * 2 + g
                        bsi = pbias.tile([1, 512], BF16, tag="bias")
                        nc.sync.dma_start(out=bsi, in_=gsibd.ap()[:, j * 512:(j + 1) * 512])
                        nc.tensor.matmul(pci_ps[32 * g:32 * g + 16, :], lhsT=ones_b[0:1, 0:16],
                                         rhs=bsi, tile_position=(0, 32 * g),
                                         start=False, stop=True, skip_group_check=True)
                    for g in range(2):
                        j = jp * 2 + g
                        if j < 8:
                            nc.vector.scalar_tensor_tensor(
                                g_sb[:, j, :], pci_ps[32 * g:32 * g + 16, :], 1.0 / WS3,
                                gh_src[:, j, :], op0=ALU.mult, op1=ALU.add)
                        else:
                            nc.scalar.activation(gn_sb[:, j - 8, :], pci_ps[32 * g:32 * g + 16, :],
                                                 AF.Copy, scale=1.0 / WS3)

            def s_elementwise(gh_src, hprev, outt):
                for t in range(4):
                    cols = slice(t * 512, (t + 1) * 512)
                    r1 = pc1.tile([FPC, 512], F32, tag="c_r")
                    nc.scalar.activation(r1, g_sb[:, t, :], AF.Sigmoid)
                    z1 = pc1.tile([FPC, 512], F32, tag="c_z")
                    nc.scalar.activation(z1, g_sb[:, 4 + t, :], AF.Sigmoid)
                    u1 = pct.tile([FPC, 512], F32, tag="cu")
                    nc.vector.tensor_tensor(u1, r1, gh_src[:, 8 + t, :], op=ALU.mult)
                    u2 = pct.tile([FPC, 512], F32, tag="cu")
                    nc.vector.tensor_tensor(u2, u1, gn_sb[:, t, :], op=ALU.add)
                    n1 = pc1.tile([FPC, 512], F32, tag="c_n")
                    nc.scalar.activation(n1, u2, AF.Tanh)
                    u3 = pct.tile([FPC, 512], F32, tag="cu")
                    nc.vector.tensor_tensor(u3, hprev[:, cols], n1, op=ALU.subtract)
                    u4 = pct.tile([FPC, 512], F32, tag="cu")
                    nc.vector.tensor_tensor(u4, z1, u3, op=ALU.mult)
                    nc.vector.tensor_tensor(outt[:, cols], n1, u4, op=ALU.add)

            gi_pass(ah_sb, gh1_sb)
            s_elementwise(gh1_sb, sc4rm_sb, s1_sb)
            for c in range(KC):
                ptp = pctps.tile([128, 16], BF16, tag="tp")
                nc.tensor.transpose(ptp, s1_sb[:, c * 128:(c + 1) * 128], ident16)
                nc.scalar.copy(s1t_sb[:, c, :], ptp)
            gi_pass(s1t_sb, gh2_sb)
            s_elementwise(gh2_sb, sfrm_sb, out32)
            nc.sync.dma_start(out=outp.ap(), in_=out32)

    nc.compile()
    return nc


def _prep_in_maps(inputs):
    E = np.ascontiguousarray(inputs["H_O_edges"].reshape(NFRAMES, ROWS, D))
    On = inputs["O_nodes"].reshape(NFRAMES, O, D)
    Hn = inputs["H_nodes"].reshape(NFRAMES, H, D)
    Sc4 = inputs["S_node_C4"].reshape(NFRAMES, D)
    Sf = np.ascontiguousarray(inputs["final_S_node"].transpose(0, 2, 1)).reshape(NFRAMES, D)

    We, Wl1, Wn = inputs["We"], inputs["Wl1"], inputs["Wn"]
    gsh_dt = NE3 if GSH_E3 else NB
    gsh_s = WS3 if GSH_E3 else 1.0

    shared = {
        "wcat": np.ascontiguousarray(
            (np.concatenate([We, Wl1], axis=0) * WS).T).astype(NE4),
        "wl1l": np.ascontiguousarray((Wl1[:, :D // 2] * WS).T).astype(NE4),
        "wl1r": np.ascontiguousarray((Wl1[:, D // 2:] * WS).T).astype(NE4),
        "wnt": np.ascontiguousarray((Wn * WS).T).astype(NE4),
        "wl2": np.ascontiguousarray(inputs["Wl2"][0].reshape(8, 128).T).astype(NB),
        "bl1t": np.ascontiguousarray(inputs["bl1"].reshape(8, 128).T).astype(NB),
        "bet8": (inputs["be"] * WS)[None, :].astype(NB),
        "bnt8": (inputs["bn"] * WS)[None, :].astype(NB),
        "pmat": np.ascontiguousarray(np.kron(np.eye(FPC), np.ones((H, 1))) / H).astype(NB),
        "ghi": np.ascontiguousarray((inputs["gh_wih"] * (WS / O)).T).astype(NE4),
        "ghh": np.ascontiguousarray((inputs["gh_whh"] * WS).T).astype(NE4),
        "ghib": (inputs["gh_bih"] * WS)[None, :].astype(NB),
        "ghhb": (inputs["gh_bhh"] * WS)[None, :].astype(NB),
        "gsi": np.ascontiguousarray((inputs["gs_wih"] * WS3).T).astype(NE3),
        "gsh": np.ascontiguousarray((inputs["gs_whh"] * gsh_s).T).astype(gsh_dt),
        "gsib": (inputs["gs_bih"] * WS3)[None, :].astype(NB),
        "gshb": (inputs["gs_bhh"] * gsh_s)[None, :].astype(NB),
    }

    in_maps = []
    for c in range(NCORES):
        fr = slice(c * FPC, (c + 1) * FPC)
        Ec = E[fr]  # [16, 128, 2048]
        e0t = np.ascontiguousarray(
            Ec.reshape(NQ, 4, ROWS, D).transpose(0, 3, 1, 2).reshape(NQ, D, 512)).astype(NE4)
        m = dict(shared)
        m.update({
            "e0t": e0t,
            "ot": np.ascontiguousarray(On[fr].reshape(FPC * O, D).T).astype(NE4),
            "ht8": np.ascontiguousarray(Hn[fr].reshape(FPC * H, D).T).astype(NE4),
            "h_rm": np.ascontiguousarray(Hn[fr].reshape(FPC * H, D)).astype(np.float32),
            "scsf": np.ascontiguousarray(np.concatenate(
                [Sc4[fr].T, Sf[fr].T], axis=1)).astype(NB),
            "sc4rm": np.ascontiguousarray(Sc4[fr]).astype(np.float32),
            "sfrm": np.ascontiguousarray(Sf[fr]).astype(np.float32),
        })
        in_maps.append(m)
    return in_maps


LAST_RESULT = None


def kernel(**inputs):
    global LAST_RESULT
    if "nc" not in _CACHE:
        _CACHE["nc"] = _build_nc()
    nc = _CACHE["nc"]
    in_maps = _prep_in_maps(inputs)
    trace = os.environ.get("KERNEL_TRACE", "0") == "1"
    res = bass_utils.run_bass_kernel_spmd(
        nc, in_maps, core_ids=list(range(NCORES)), trace=trace)
    LAST_RESULT = res
    out = np.concatenate([res.results[c]["outp"] for c in range(NCORES)], axis=0)
    return np.ascontiguousarray(out.reshape(B, F, D)).astype(np.float32)


# revision 11
# speedup vs baseline: 1.5038x; 1.2110x over previous
"""Trainium2 Bass kernel for nn_Graph_Enhance_model (GNN message passing).

Self-contained: hardcodes shapes B=4,F=32,H=8,O=16,D=2048, 8 cores.
Data-parallel over the 128 (b,f) frames: 16 frames per core.

v2: algebraic step-1 restructure (E1 = w0*[me0;mn] is rank-structured, so
step-1 waves collapse to half-K matmuls on me0 plus per-o weighted
reductions folded before the We matmul), fp8e4m3 DoubleRow for the big
matmuls, fp8 weights for the human GRU, e3m4 wih / bf16 whh for the
S-node GRUs, col-group-packed small-M matmuls in the S-GRU phase.
"""

import os
import sys

for _p in ("/opt/trn_rl_repo", "/opt/pypackages"):
    if _p not in sys.path and os.path.isdir(_p):
        sys.path.append(_p)

import numpy as np
import ml_dtypes

import concourse.bass as bass
import concourse.bacc as bacc
import concourse.tile as tile
import concourse.mybir as mybir
from concourse import bass_utils
from concourse.masks import make_identity

BF16 = mybir.dt.bfloat16
F32 = mybir.dt.float32
F8 = mybir.dt.float8e4
F8E3 = mybir.dt.float8e3
AF = mybir.ActivationFunctionType
ALU = mybir.AluOpType
AX = mybir.AxisListType
DR = mybir.MatmulPerfMode.DoubleRow

NB = ml_dtypes.bfloat16
NE4 = ml_dtypes.float8_e4m3
NE3 = ml_dtypes.float8_e3m4

B, F, H, O, D = 4, 32, 8, 16, 2048
NFRAMES = B * F          # 128
NCORES = 8
FPC = NFRAMES // NCORES  # 16 frames per core
ROWS = H * O             # 128 rows per frame
KC = D // 128            # 16 K-chunks
NQ = FPC // 4            # 4 quads of 4 frames

WS = 8.0                 # fp8e4 weight scale
WS3 = 64.0               # e3m4 weight scale
GSH_E3 = False           # S-GRU whh in e3m4 (True) or bf16 (False)

_CACHE = {}


def _bc4(t, kc, q):
    """Broadcast-over-h AP: [128, 4f, 8h(stride0), 16o] of t[:, kc, q*64:(q+1)*64]."""
    base = t[:, kc, q * 64:(q + 1) * 64]
    return bass.AP(tensor=base.tensor, offset=base.offset,
                   ap=[list(base.ap[0]), [16, 4], [0, 8], [1, 16]])


def _r4(t):
    """[128, 512] -> [128, 4f, 8h, 16o]."""
    return t.rearrange("p (f h o) -> p f h o", f=4, h=8)


def _build_nc():
    nc = bacc.Bacc("TRN2", target_bir_lowering=False, debug=False, num_devices=NCORES)

    def din(name, shape, dt):
        return nc.dram_tensor(name, shape, dt, kind="ExternalInput")

    e0t = din("e0t", [NQ, D, 512], F8)
    ot = din("ot", [D, FPC * O], F8)
    wnt = din("wnt", [D, D // 2], F8)
    wcat = din("wcat", [D, D], F8)
    wl1l = din("wl1l", [D // 2, D // 2], F8)
    wl1r = din("wl1r", [D // 2, D // 2], F8)
    wl2 = din("wl2", [128, 8], BF16)
    bl1td = din("bl1t", [128, 8], BF16)
    bettd = din("bett", [128, 8], BF16)
    bnttd = din("bntt", [128, 8], BF16)
    hindd = din("hind", [128, 512], BF16)
    ht8d = din("ht8", [D, FPC * H], F8)
    h_rmd = din("h_rm", [FPC * H, D], F32)
    pmatd = din("pmat", [FPC * H, FPC], BF16)
    ghi = din("ghi", [D, 3 * D], F8)
    ghh = din("ghh", [D, 3 * D], F8)
    ghibd = din("ghib", [1, 3 * D], BF16)
    ghhbd = din("ghhb", [1, 3 * D], BF16)
    gsid = din("gsi", [D, 3 * D], F8E3)
    gshd = din("gsh", [D, 3 * D], F8E3 if GSH_E3 else BF16)
    gsibd = din("gsib", [1, 3 * D], BF16)
    gshbd = din("gshb", [1, 3 * D], BF16)
    scsfd = din("scsf", [D, 2 * FPC], BF16)
    sc4rmd = din("sc4rm", [FPC, D], F32)
    sfrmd = din("sfrm", [FPC, D], F32)
    outp = nc.dram_tensor("outp", [FPC, D], F32, kind="ExternalOutput")

    SH = 1.0 / WS3 if GSH_E3 else 1.0   # descale for gsh-side psums
    GSH_DT = F8E3 if GSH_E3 else BF16

    from contextlib import ExitStack

    with tile.TileContext(nc) as tc, ExitStack() as ctx:
        glob = ctx.enter_context(tc.tile_pool(name="glob", bufs=1))
        pbias = ctx.enter_context(tc.tile_pool(name="pbias", bufs=3, side="right"))

        oi_t = glob.tile([16, 544], BF16)
        ident16 = oi_t[0:16, 0:16]
        make_identity(nc, ident16)
        ones_b = oi_t[0:1, 32:544]
        nc.vector.memset(ones_b, 1.0)
        wb_t = glob.tile([128, 32], BF16)
        wl2_sb = wb_t[:, 0:8]
        nc.sync.dma_start(out=wl2_sb, in_=wl2.ap())
        bl1t_sb = wb_t[:, 8:16]
        nc.sync.dma_start(out=bl1t_sb, in_=bl1td.ap())
        bett_sb = wb_t[:, 16:24]
        nc.sync.dma_start(out=bett_sb, in_=bettd.ap())
        bntt_sb = wb_t[:, 24:32]
        nc.sync.dma_start(out=bntt_sb, in_=bnttd.ap())
        hind_sb = glob.tile([128, 512], BF16)
        nc.sync.dma_start(out=hind_sb, in_=hindd.ap())

        f8pair = glob.tile([128, KC, 2 * FPC * H], F8)
        msum_f8 = f8pair[:, :, 0:FPC * H]
        ht8_sb = f8pair[:, :, FPC * H:2 * FPC * H]
        nc.sync.dma_start(out=ht8_sb, in_=ht8d.ap().rearrange("(kc p) n -> p kc n", p=128))
        bfpack = glob.tile([128, KC, 4 * FPC], BF16)
        scsf_sb = bfpack[:, :, 0:2 * FPC]
        nc.sync.dma_start(out=scsf_sb, in_=scsfd.ap().rearrange("(kc p) n -> p kc n", p=128))
        ah_sb = bfpack[:, :, 2 * FPC:3 * FPC]
        s1t_sb = bfpack[:, :, 3 * FPC:4 * FPC]

        bw_tiles = {}
        bw_order = []
        for t in range(4):
            bw_order += [("h", t), ("i", t), ("h", 4 + t), ("i", 4 + t),
                         ("i", 8 + t), ("h", 8 + t)]

        with tc.tile_pool(name="bw", bufs=3, side="right") as bwpool:

            def bw_load(src, j):
                wt = bwpool.tile([128, KC, 512], F8, tag="bw")
                mat = ghh if src == "h" else ghi
                nc.sync.dma_start(out=wt, in_=mat.ap()[:, j * 512:(j + 1) * 512]
                                  .rearrange("(kc p) m -> p kc m", p=128))
                bw_tiles[(src, j)] = wt

            # ================= Phase A =================
            with (
                tc.tile_pool(name="pal", bufs=1) as pal,
                tc.tile_pool(name="pwcat", bufs=1) as pwcat,
                tc.tile_pool(name="pa", bufs=2) as pa,
                tc.tile_pool(name="pam", bufs=2) as pam,
                tc.tile_pool(name="prelu", bufs=2) as prelu,
                tc.tile_pool(name="pa1", bufs=1) as pa1,
                tc.tile_pool(name="pav", bufs=2) as pav,
            ):
                mn_f8 = pal.tile([128, 8, FPC * O], F8)       # mn^T, unscaled
                q8rm = pal.tile([128, 2, D // 2], BF16)       # 8 * (Wl1R mn), row-major
                xu_f = pal.tile([128, KC, FPC * H], F32)      # (me0u ⊕ mnu)^T
                xu_b = pal.tile([128, KC, FPC * H], BF16)
                msum_f = pal.tile([128, KC, FPC * H], F32)    # msum^T (raw sum over o)

                wcat_sb = pwcat.tile([128, KC, D], F8)
                nc.sync.dma_start(out=wcat_sb, in_=wcat.ap().rearrange("(kc p) m -> p kc m", p=128))
                wl1l_sb = pwcat.tile([128, 8, D // 2], F8)
                nc.sync.dma_start(out=wl1l_sb, in_=wl1l.ap().rearrange("(kc p) m -> p kc m", p=128))

                # ---- Phase 0: mn^T = Wn O^T + bn; Q = Wl1R mn ----
                with (
                    tc.tile_pool(name="p0", bufs=1) as p0,
                    tc.tile_pool(name="p0ps", bufs=4, space="PSUM") as p0ps,
                ):
                    wnt_sb = p0.tile([128, KC, D // 2], F8)
                    nc.sync.dma_start(out=wnt_sb, in_=wnt.ap().rearrange("(kc p) m -> p kc m", p=128))
                    ot_sb = p0.tile([128, KC, FPC * O], F8)
                    nc.sync.dma_start(out=ot_sb, in_=ot.ap().rearrange("(kc p) n -> p kc n", p=128))
                    wl1r_sb = p0.tile([128, 8, D // 2], F8)
                    nc.sync.dma_start(out=wl1r_sb, in_=wl1r.ap().rearrange("(kc p) m -> p kc m", p=128))
                    for mt in range(8):
                        pm = p0ps.tile([128, FPC * O], F32, tag="pm")
                        for i in range(8):
                            nc.tensor.matmul(pm, lhsT=wnt_sb[:, 2 * i:2 * i + 2, mt * 128:(mt + 1) * 128],
                                             rhs=ot_sb[:, 2 * i:2 * i + 2, :],
                                             perf_mode=DR, start=(i == 0), stop=(i == 7))
                        nc.scalar.activation(mn_f8[:, mt, :], pm, AF.Identity,
                                             bias=bntt_sb[:, mt:mt + 1], scale=1.0 / WS)
                    # Q row-major: q8rm[fo-chunk c] = 8 * (mn @ Wl1R.T)
                    for c in range(2):
                        for n in range(2):
                            pq = p0ps.tile([128, 512], F32, tag="pq")
                            for i in range(4):
                                nc.tensor.matmul(pq, lhsT=mn_f8[:, 2 * i:2 * i + 2, c * 128:(c + 1) * 128],
                                                 rhs=wl1r_sb[:, 2 * i:2 * i + 2, n * 512:(n + 1) * 512],
                                                 perf_mode=DR, start=(i == 0), stop=(i == 3))
                            nc.scalar.copy(q8rm[:, c, n * 512:(n + 1) * 512], pq)

                with tc.tile_pool(name="paps", bufs=4, space="PSUM") as paps, \
                     tc.tile_pool(name="papss", bufs=2, space="PSUM") as papss:

                    def softmax_block(relu_t, wtag):
                        pl = papss.tile([1, 512], F32, tag="pl")
                        for kc2 in range(8):
                            nc.tensor.matmul(pl, lhsT=wl2_sb[:, kc2:kc2 + 1],
                                             rhs=relu_t[:, kc2, :], start=(kc2 == 0), stop=(kc2 == 7))
                        pl3 = pl.rearrange("o (g i) -> o g i", i=16)
                        smt = pa1.tile([1, 96], F32, tag="smt")
                        mx, sm, rs = smt[:, 0:32], smt[:, 32:64], smt[:, 64:96]
                        nc.vector.reduce_max(mx, pl3, axis=AX.X)
                        sub = pa1.tile([1, 512], F32, tag="sub")
                        nc.vector.tensor_tensor(sub.rearrange("o (g i) -> o g i", i=16), pl3,
                                                mx.broadcast_to((1, 32, 16)), op=ALU.subtract)
                        nc.scalar.activation(sub, sub, AF.Exp)
                        ex3 = sub.rearrange("o (g i) -> o g i", i=16)
                        nc.vector.reduce_sum(sm, ex3, axis=AX.X)
                        nc.vector.reciprocal(rs, sm)
                        w_sb = wrow[:, 0 if wtag == "w0" else 1, :]
                        nc.vector.tensor_tensor(w_sb.rearrange("o (g i) -> o g i", i=16), ex3,
                                                rs.broadcast_to((1, 32, 16)), op=ALU.mult)
                        return w_sb

                    def broadcast_w(w_sb, bidx):
                        pw = papss.tile([128, 512], F32, tag="pw")
                        nc.tensor.matmul(pw, lhsT=ones_b[0:1, 0:128], rhs=w_sb,
                                         start=True, stop=True)
                        wb = wbb[:, bidx, :]
                        nc.scalar.copy(wb, pw)
                        return wb

                    for q in range(NQ):
                        xq = pa.tile([128, KC, 512], F8, tag="xq")
                        nc.sync.dma_start(out=xq, in_=e0t.ap()[q].rearrange("(kc p) n -> p kc n", p=128))
                        me0t = pam.tile([128, 8, 512], F8, tag="me0t")
                        relu_sb = prelu.tile([128, 8, 512], BF16, tag="relu")
                        wrow = pa1.tile([1, 2, 512], BF16, tag="wrow")
                        wbb = pa1.tile([128, 3, 512], BF16, tag="wbb")

                        # step0: me0 = We E + be
                        for mt in range(8):
                            pe = paps.tile([128, 512], F32, tag="wave")
                            for i in range(8):
                                nc.tensor.matmul(pe, lhsT=wcat_sb[:, 2 * i:2 * i + 2, mt * 128:(mt + 1) * 128],
                                                 rhs=xq[:, 2 * i:2 * i + 2, :],
                                                 perf_mode=DR, start=(i == 0), stop=(i == 7))
                            nc.scalar.activation(me0t[:, mt, :], pe, AF.Identity,
                                                 bias=bett_sb[:, mt:mt + 1], scale=1.0 / WS)
                        # step0: a0 = relu(Wl1 E + bl1)
                        for mt in range(8, 16):
                            pe = paps.tile([128, 512], F32, tag="wave")
                            for i in range(8):
                                nc.tensor.matmul(pe, lhsT=wcat_sb[:, 2 * i:2 * i + 2, mt * 128:(mt + 1) * 128],
                                                 rhs=xq[:, 2 * i:2 * i + 2, :],
                                                 perf_mode=DR, start=(i == 0), stop=(i == 7))
                            nc.scalar.activation(relu_sb[:, mt - 8, :], pe, AF.Relu,
                                                 bias=bl1t_sb[:, mt - 8:mt - 7], scale=1.0 / WS)
                        w0_sb = softmax_block(relu_sb, "w0")
                        w0b = broadcast_w(w0_sb, 0)

                        # step1: a1 = relu(w0*(P+Q) + bl1), P = Wl1L me0
                        qbase = (q % 2) * 64
                        for mt in range(8):
                            pp = paps.tile([128, 512], F32, tag="wave")
                            for i in range(4):
                                nc.tensor.matmul(pp, lhsT=wl1l_sb[:, 2 * i:2 * i + 2, mt * 128:(mt + 1) * 128],
                                                 rhs=me0t[:, 2 * i:2 * i + 2, :],
                                                 perf_mode=DR, start=(i == 0), stop=False)
                            nc.tensor.matmul(pp, lhsT=q8rm[qbase:qbase + 64, q // 2, mt * 128:(mt + 1) * 128],
                                             rhs=hind_sb[qbase:qbase + 64, :],
                                             start=False, stop=True)
                            v2 = pav.tile([128, 512], BF16, tag="v")
                            nc.vector.tensor_tensor(v2, pp, w0b, op=ALU.mult)
                            nc.scalar.activation(relu_sb[:, mt, :], v2, AF.Relu,
                                                 bias=bl1t_sb[:, mt:mt + 1], scale=1.0 / WS)
                        w1_sb = softmax_block(relu_sb, "w1")
                        w1b = broadcast_w(w1_sb, 1)
                        ub = wbb[:, 2, :]
                        nc.vector.tensor_tensor(ub, w0b, w1b, op=ALU.mult)

                        # weighted reductions over o
                        for kc in range(8):
                            tmp = pav.tile([128, 512], BF16, tag="tmp")
                            nc.vector.tensor_tensor(tmp, me0t[:, kc, :], ub, op=ALU.mult)
                            nc.vector.reduce_sum(xu_f[:, kc, q * 32:(q + 1) * 32], _r4(tmp), axis=AX.X)
                        for kc in range(8):
                            tmp = pav.tile([128, 512], BF16, tag="tmp")
                            nc.vector.tensor_tensor(_r4(tmp), _bc4(mn_f8, kc, q), _r4(ub), op=ALU.mult)
                            nc.vector.reduce_sum(xu_f[:, 8 + kc, q * 32:(q + 1) * 32], _r4(tmp), axis=AX.X)
                            tmp2 = pav.tile([128, 512], BF16, tag="tmp")
                            nc.vector.tensor_tensor(_r4(tmp2), _bc4(mn_f8, kc, q), _r4(w1b), op=ALU.mult)
                            nc.vector.reduce_sum(msum_f[:, 8 + kc, q * 32:(q + 1) * 32], _r4(tmp2), axis=AX.X)

                        if q == 0:  # prefetch first GRU weight blocks under phase A
                            for src, j in bw_order[:3]:
                                bw_load(src, j)

                    for kc in range(KC):
                        nc.vector.tensor_copy(xu_b[:, kc, :], xu_f[:, kc, :])
                    # folded msum_e = We (me0u ⊕ mnu) + be
                    for mt in range(8):
                        pf = papss.tile([128, FPC * H], F32, tag="pw")
                        for kc in range(KC):
                            nc.tensor.matmul(pf, lhsT=wcat_sb[:, kc, mt * 128:(mt + 1) * 128],
                                             rhs=xu_b[:, kc, :], start=(kc == 0), stop=(kc == KC - 1))
                        nc.scalar.activation(msum_f[:, mt, :], pf, AF.Identity,
                                             bias=bett_sb[:, mt:mt + 1], scale=1.0 / WS)
                    for kc in range(KC):
                        nc.vector.tensor_copy(msum_f8[:, kc, :], msum_f[:, kc, :])

            # ============ Phase B (with PH pass interleaved) ============
            with tc.tile_pool(name="pcg1", bufs=1) as pcg1:
                gh1_sb = pcg1.tile([FPC, 12, 512], BF16)   # whh Sc4 + bhh (unscaled)
                gh2_sb = pcg1.tile([FPC, 12, 512], BF16)   # whh Sf + bhh
                with (
                    tc.tile_pool(name="pcw", bufs=2, side="right") as pcw,
                    tc.tile_pool(name="pchps", bufs=1, space="PSUM") as pchps,
                    tc.tile_pool(name="pbh", bufs=1) as pbh,
                    tc.tile_pool(name="pbt", bufs=2) as pbt,
                    tc.tile_pool(name="pbps", bufs=1, space="PSUM") as pbps,
                    tc.tile_pool(name="pbps2", bufs=2, space="PSUM") as pbps2,
                ):
                    NR = FPC * H  # 128 rows
                    h_rm = pbh.tile([NR, D], F32)
                    nc.sync.dma_start(out=h_rm, in_=h_rmd.ap())
                    pmat_sb = pbh.tile([NR, FPC], BF16)
                    nc.sync.dma_start(out=pmat_sb, in_=pmatd.ap())
                    hum_b = pbh.tile([NR, D], BF16)

                    def ph_pack(jp):
                        """gh1/gh2 = whh [Sc4|Sf] + bhh for j-blocks jp*2, jp*2+1."""
                        pch1 = pchps.tile([128, 512], F32, tag="pch1")
                        pch2 = pchps.tile([128, 512], F32, tag="pch2")
                        wts = []
                        for g in range(2):
                            j = jp * 2 + g
                            wt = pcw.tile([128, KC, 512], GSH_DT, tag="cw")
                            nc.sync.dma_start(out=wt, in_=gshd.ap()[:, j * 512:(j + 1) * 512]
                                              .rearrange("(kc p) m -> p kc m", p=128))
                            wts.append(wt)
                        for kc in range(KC):
                            for g in range(2):
                                nc.tensor.matmul(pch1[32 * g:32 * g + 16, :],
                                                 lhsT=scsf_sb[:, kc, 0:FPC], rhs=wts[g][:, kc, :],
                                                 tile_position=(0, 32 * g),
                                                 start=(kc == 0), stop=False, skip_group_check=True)
                                nc.tensor.matmul(pch2[32 * g:32 * g + 16, :],
                                                 lhsT=scsf_sb[:, kc, FPC:2 * FPC], rhs=wts[g][:, kc, :],
                                                 tile_position=(0, 32 * g),
                                                 start=(kc == 0), stop=False, skip_group_check=True)
                        for g in range(2):
                            j = jp * 2 + g
                            bsh = pbias.tile([1, 512], BF16, tag="bias")
                            nc.sync.dma_start(out=bsh, in_=gshbd.ap()[:, j * 512:(j + 1) * 512])
                            nc.tensor.matmul(pch1[32 * g:32 * g + 16, :], lhsT=ones_b[0:1, 0:16],
                                             rhs=bsh, tile_position=(0, 32 * g),
                                             start=False, stop=True, skip_group_check=True)
                            nc.tensor.matmul(pch2[32 * g:32 * g + 16, :], lhsT=ones_b[0:1, 0:16],
                                             rhs=bsh, tile_position=(0, 32 * g),
                                             start=False, stop=True, skip_group_check=True)
                        for g in range(2):
                            j = jp * 2 + g
                            nc.scalar.activation(gh1_sb[:, j, :], pch1[32 * g:32 * g + 16, :],
                                                 AF.Copy, scale=SH)
                            nc.scalar.activation(gh2_sb[:, j, :], pch2[32 * g:32 * g + 16, :],
                                                 AF.Copy, scale=SH)

                    def gh_block(j, pt, use_i, use_h):
                        ops = []
                        if use_h:
                            ops += [("h", j, i) for i in range(8)] + [(ghhbd, None, None)]
                        if use_i:
                            ops += [("i", j, i) for i in range(8)] + [(ghibd, None, None)]
                        for idx, (src, jj, i) in enumerate(ops):
                            st, sp = idx == 0, idx == len(ops) - 1
                            if jj is None:
                                bb = pbias.tile([1, 512], BF16, tag="bias")
                                nc.sync.dma_start(out=bb, in_=src.ap()[:, j * 512:(j + 1) * 512])
                                nc.tensor.matmul(pt, lhsT=ones_b[0:1, 0:NR], rhs=bb,
                                                 start=st, stop=sp)
                            else:
                                if (src, jj) not in bw_tiles:
                                    bw_load(src, jj)
                                wt = bw_tiles[(src, jj)]
                                x = ht8_sb if src == "h" else msum_f8
                                nc.tensor.matmul(pt, lhsT=x[:, 2 * i:2 * i + 2, :],
                                                 rhs=wt[:, 2 * i:2 * i + 2, :],
                                                 perf_mode=DR, start=st, stop=sp)

                    for t in range(4):
                        cols = slice(t * 512, (t + 1) * 512)
                        p_r = pbps.tile([NR, 512], F32, tag="pr")
                        gh_block(t, p_r, True, True)
                        p_z = pbps.tile([NR, 512], F32, tag="pz")
                        gh_block(4 + t, p_z, True, True)
                        p_in = pbps.tile([NR, 512], F32, tag="pin")
                        gh_block(8 + t, p_in, True, False)
                        p_hn = pbps.tile([NR, 512], F32, tag="phn")
                        gh_block(8 + t, p_hn, False, True)
                        r_sb = pbh.tile([NR, 512], F32, tag="r")
                        nc.scalar.activation(r_sb, p_r, AF.Sigmoid, scale=1.0 / WS)
                        z_sb = pbh.tile([NR, 512], F32, tag="z")
                        nc.scalar.activation(z_sb, p_z, AF.Sigmoid, scale=1.0 / WS)
                        t1 = pbt.tile([NR, 512], F32, tag="tt")
                        nc.vector.tensor_tensor(t1, r_sb, p_hn, op=ALU.mult)
                        t2 = pbt.tile([NR, 512], F32, tag="tt")
                        nc.vector.tensor_tensor(t2, t1, p_in, op=ALU.add)
                        n_sb = pbh.tile([NR, 512], F32, tag="n")
                        nc.scalar.activation(n_sb, t2, AF.Tanh, scale=1.0 / WS)
                        t3 = pbt.tile([NR, 512], F32, tag="tt")
                        nc.vector.tensor_tensor(t3, h_rm[:, cols], n_sb, op=ALU.subtract)
                        t4 = pbt.tile([NR, 512], F32, tag="tt")
                        nc.vector.tensor_tensor(t4, z_sb, t3, op=ALU.mult)
                        nc.vector.tensor_tensor(hum_b[:, cols], n_sb, t4, op=ALU.add)
                        if t < 3:
                            ph_pack(2 * t)
                            ph_pack(2 * t + 1)
                    for c in range(KC):
                        pah = pbps2.tile([128, FPC], F32, tag="pah")
                        nc.tensor.matmul(pah, lhsT=hum_b[:, c * 128:(c + 1) * 128], rhs=pmat_sb,
                                         start=True, stop=True)
                        nc.scalar.copy(ah_sb[:, c, :], pah)

                # ============ Phase C: two S-node GRUs ============
                with (
                    tc.tile_pool(name="pc1", bufs=1) as pc1,
                    tc.tile_pool(name="pct", bufs=2) as pct,
                    tc.tile_pool(name="pci", bufs=3, side="right") as pci,
                    tc.tile_pool(name="pcps", bufs=2, space="PSUM") as pcps,
                    tc.tile_pool(name="pctps", bufs=2, space="PSUM") as pctps,
                ):
                    sc4rm_sb = pc1.tile([FPC, D], F32)
                    nc.sync.dma_start(out=sc4rm_sb, in_=sc4rmd.ap())
                    sfrm_sb = pc1.tile([FPC, D], F32)
                    nc.sync.dma_start(out=sfrm_sb, in_=sfrmd.ap())
                    g_sb = pc1.tile([FPC, 8, 512], BF16, tag="g")     # r,z gates
                    gn_sb = pc1.tile([FPC, 4, 512], BF16, tag="gn")   # inn
                    s1_sb = pc1.tile([FPC, D], BF16)
                    out32 = pc1.tile([FPC, D], F32)

                    def gi_pass(xt, gh_src):
                        """gi = wih x + bih (x64); g = gi/64 + gh for r,z; gi/64 for n."""
                        for jp in range(6):
                            pci_ps = pcps.tile([128, 512], F32, tag="pch")
                            wts = []
                            for g in range(2):
                                j = jp * 2 + g
                                wt = pci.tile([128, KC, 512], F8E3, tag="ci")
                                nc.sync.dma_start(out=wt, in_=gsid.ap()[:, j * 512:(j + 1) * 512]
                                                  .rearrange("(kc p) m -> p kc m", p=128))
                                wts.append(wt)
                            for kc in range(KC):
                                for g in range(2):
                                    nc.tensor.matmul(pci_ps[32 * g:32 * g + 16, :],
                                                     lhsT=xt[:, kc, :], rhs=wts[g][:, kc, :],
                                                     tile_position=(0, 32 * g),
                                                     start=(kc == 0), stop=False, skip_group_check=True)
                            for g in range(2):
                                j = jp * 2 + g
                                bsi = pbias.tile([1, 512], BF16, tag="bias")
                                nc.sync.dma_start(out=bsi, in_=gsibd.ap()[:, j * 512:(j + 1) * 512])
                                nc.tensor.matmul(pci_ps[32 * g:32 * g + 16, :], lhsT=ones_b[0:1, 0:16],
                                                 rhs=bsi, tile_position=(0, 32 * g),
                                                 start=False, stop=True, skip_group_check=True)
                            for g in range(2):
                                j = jp * 2 + g
                                if j < 8:
                                    nc.vector.scalar_tensor_tensor(
                                        g_sb[:, j, :], pci_ps[32 * g:32 * g + 16, :], 1.0 / WS3,
                                        gh_src[:, j, :], op0=ALU.mult, op1=ALU.add)
                                else:
                                    nc.scalar.activation(gn_sb[:, j - 8, :], pci_ps[32 * g:32 * g + 16, :],
                                                         AF.Copy, scale=1.0 / WS3)

                    def s_elementwise(gh_src, hprev, outt):
                        for t in range(4):
                            cols = slice(t * 512, (t + 1) * 512)
                            r1 = pc1.tile([FPC, 512], F32, tag="c_r")
                            nc.scalar.activation(r1, g_sb[:, t, :], AF.Sigmoid)
                            z1 = pc1.tile([FPC, 512], F32, tag="c_z")
                            nc.scalar.activation(z1, g_sb[:, 4 + t, :], AF.Sigmoid)
                            u1 = pct.tile([FPC, 512], F32, tag="cu")
                            nc.vector.tensor_tensor(u1, r1, gh_src[:, 8 + t, :], op=ALU.mult)
                            u2 = pct.tile([FPC, 512], F32, tag="cu")
                            nc.vector.tensor_tensor(u2, u1, gn_sb[:, t, :], op=ALU.add)
                            n1 = pc1.tile([FPC, 512], F32, tag="c_n")
                            nc.scalar.activation(n1, u2, AF.Tanh)
                            u3 = pct.tile([FPC, 512], F32, tag="cu")
                            nc.vector.tensor_tensor(u3, hprev[:, cols], n1, op=ALU.subtract)
                            u4 = pct.tile([FPC, 512], F32, tag="cu")
                            nc.vector.tensor_tensor(u4, z1, u3, op=ALU.mult)
                            nc.vector.tensor_tensor(outt[:, cols], n1, u4, op=ALU.add)

                    gi_pass(ah_sb, gh1_sb)
                    s_elementwise(gh1_sb, sc4rm_sb, s1_sb)
                    for c in range(KC):
                        ptp = pctps.tile([128, 16], BF16, tag="tp")
                        nc.tensor.transpose(ptp, s1_sb[:, c * 128:(c + 1) * 128], ident16)
                        nc.scalar.copy(s1t_sb[:, c, :], ptp)
                    gi_pass(s1t_sb, gh2_sb)
                    s_elementwise(gh2_sb, sfrm_sb, out32)
                    nc.sync.dma_start(out=outp.ap(), in_=out32)

    nc.compile()
    return nc


def _make_hind():
    """h-broadcast indicator: hind[p, f*128+h*16+o] = (p%64 == f*16+o)."""
    m = np.zeros((128, 512), dtype=NB)
    for n in range(512):
        f, o = n // 128, n % 16
        m[f * 16 + o, n] = 1.0
        m[64 + f * 16 + o, n] = 1.0
    return m


def _prep_in_maps(inputs):
    E = np.ascontiguousarray(inputs["H_O_edges"].reshape(NFRAMES, ROWS, D))
    On = inputs["O_nodes"].reshape(NFRAMES, O, D)
    Hn = inputs["H_nodes"].reshape(NFRAMES, H, D)
    Sc4 = inputs["S_node_C4"].reshape(NFRAMES, D)
    Sf = np.ascontiguousarray(inputs["final_S_node"].transpose(0, 2, 1)).reshape(NFRAMES, D)

    We, Wl1, Wn = inputs["We"], inputs["Wl1"], inputs["Wn"]
    gsh_dt = NE3 if GSH_E3 else NB
    gsh_s = WS3 if GSH_E3 else 1.0

    shared = {
        "wcat": np.ascontiguousarray(
            (np.concatenate([We, Wl1], axis=0) * WS).T).astype(NE4),
        "wl1l": np.ascontiguousarray((Wl1[:, :D // 2] * WS).T).astype(NE4),
        "wl1r": np.ascontiguousarray((Wl1[:, D // 2:] * WS).T).astype(NE4),
        "wnt": np.ascontiguousarray((Wn * WS).T).astype(NE4),
        "wl2": np.ascontiguousarray(inputs["Wl2"][0].reshape(8, 128).T).astype(NB),
        "bl1t": np.ascontiguousarray(inputs["bl1"].reshape(8, 128).T).astype(NB),
        "bett": np.ascontiguousarray(inputs["be"].reshape(8, 128).T).astype(NB),
        "bntt": np.ascontiguousarray(inputs["bn"].reshape(8, 128).T).astype(NB),
        "hind": _make_hind(),
        "pmat": np.ascontiguousarray(np.kron(np.eye(FPC), np.ones((H, 1))) / H).astype(NB),
        "ghi": np.ascontiguousarray((inputs["gh_wih"] * (WS / O)).T).astype(NE4),
        "ghh": np.ascontiguousarray((inputs["gh_whh"] * WS).T).astype(NE4),
        "ghib": (inputs["gh_bih"] * WS)[None, :].astype(NB),
        "ghhb": (inputs["gh_bhh"] * WS)[None, :].astype(NB),
        "gsi": np.ascontiguousarray((inputs["gs_wih"] * WS3).T).astype(NE3),
        "gsh": np.ascontiguousarray((inputs["gs_whh"] * gsh_s).T).astype(gsh_dt),
        "gsib": (inputs["gs_bih"] * WS3)[None, :].astype(NB),
        "gshb": (inputs["gs_bhh"] * gsh_s)[None, :].astype(NB),
    }

    in_maps = []
    for c in range(NCORES):
        fr = slice(c * FPC, (c + 1) * FPC)
        Ec = E[fr]  # [16, 128, 2048]
        e0t = np.ascontiguousarray(
            Ec.reshape(NQ, 4, ROWS, D).transpose(0, 3, 1, 2).reshape(NQ, D, 512)).astype(NE4)
        m = dict(shared)
        m.update({
            "e0t": e0t,
            "ot": np.ascontiguousarray(On[fr].reshape(FPC * O, D).T).astype(NE4),
            "ht8": np.ascontiguousarray(Hn[fr].reshape(FPC * H, D).T).astype(NE4),
            "h_rm": np.ascontiguousarray(Hn[fr].reshape(FPC * H, D)).astype(np.float32),
            "scsf": np.ascontiguousarray(np.concatenate(
                [Sc4[fr].T, Sf[fr].T], axis=1)).astype(NB),
            "sc4rm": np.ascontiguousarray(Sc4[fr]).astype(np.float32),
            "sfrm": np.ascontiguousarray(Sf[fr]).astype(np.float32),
        })
        in_maps.append(m)
    return in_maps


LAST_RESULT = None


def kernel(**inputs):
    global LAST_RESULT
    if "nc" not in _CACHE:
        _CACHE["nc"] = _build_nc()
    nc = _CACHE["nc"]
    in_maps = _prep_in_maps(inputs)
    trace = os.environ.get("KERNEL_TRACE", "0") == "1"
    res = bass_utils.run_bass_kernel_spmd(
        nc, in_maps, core_ids=list(range(NCORES)), trace=trace)
    LAST_RESULT = res
    out = np.concatenate([res.results[c]["outp"] for c in range(NCORES)], axis=0)
    return np.ascontiguousarray(out.reshape(B, F, D)).astype(np.float32)


# revision 13
# speedup vs baseline: 1.7943x; 1.1932x over previous
"""Trainium2 Bass kernel for nn_Graph_Enhance_model (GNN message passing).

Self-contained: hardcodes shapes B=4,F=32,H=8,O=16,D=2048, 8 cores.
Data-parallel over the 128 (b,f) frames: 16 frames per core.

v4: algebraic step-1 restructure, fp8e4m3 DoubleRow waves, fp8 human-GRU
weights, e3m4 S-GRU weights, pre-tiled weight DRAM layouts (contiguous
DMA bursts), dual HWDGE queues (sync + scalar), whh-gates computed during
phase A, PH pass interleaved with phase B, col-group-packed small-M
matmuls, half-resident gsi across both S-GRU steps.
"""

import os
import sys

for _p in ("/opt/trn_rl_repo", "/opt/pypackages"):
    if _p not in sys.path and os.path.isdir(_p):
        sys.path.append(_p)

import numpy as np
import ml_dtypes

import concourse.bass as bass
import concourse.bacc as bacc
import concourse.tile as tile
import concourse.mybir as mybir
from concourse import bass_utils
from concourse.masks import make_identity

BF16 = mybir.dt.bfloat16
F32 = mybir.dt.float32
F8 = mybir.dt.float8e4
F8E3 = mybir.dt.float8e3
AF = mybir.ActivationFunctionType
ALU = mybir.AluOpType
AX = mybir.AxisListType
DR = mybir.MatmulPerfMode.DoubleRow

NB = ml_dtypes.bfloat16
NE4 = ml_dtypes.float8_e4m3
NE3 = ml_dtypes.float8_e3m4

B, F, H, O, D = 4, 32, 8, 16, 2048
NFRAMES = B * F          # 128
NCORES = 8
FPC = NFRAMES // NCORES  # 16 frames per core
ROWS = H * O             # 128 rows per frame
KC = D // 128            # 16 K-chunks
NQ = FPC // 4            # 4 quads of 4 frames

WS = 8.0                 # fp8e4 weight scale
WS3 = 64.0               # e3m4 weight scale

_CACHE = {}


def _bc4(t, kc, q):
    """Broadcast-over-h AP: [128, 4f, 8h(stride0), 16o] of t[:, kc, q*64:(q+1)*64]."""
    base = t[:, kc, q * 64:(q + 1) * 64]
    return bass.AP(tensor=base.tensor, offset=base.offset,
                   ap=[list(base.ap[0]), [16, 4], [0, 8], [1, 16]])


def _r4(t):
    """[128, 512] -> [128, 4f, 8h, 16o]."""
    return t.rearrange("p (f h o) -> p f h o", f=4, h=8)


def _build_nc():
    nc = bacc.Bacc("TRN2", target_bir_lowering=False, debug=False, num_devices=NCORES)

    def din(name, shape, dt):
        return nc.dram_tensor(name, shape, dt, kind="ExternalInput")

    e0t = din("e0t", [NQ, 128, KC, 512], F8)
    ot = din("ot", [128, KC, FPC * O], F8)
    wnt = din("wnt", [128, KC, D // 2], F8)
    wcatA = din("wcatA", [128, KC, D // 2], F8)   # We^T (x8)
    wcatB = din("wcatB", [128, KC, D // 2], F8)   # Wl1^T (x8)
    wl1l = din("wl1l", [128, 8, D // 2], F8)
    wl1r = din("wl1r", [128, 8, D // 2], F8)
    wl2 = din("wl2", [128, 8], BF16)
    bl1td = din("bl1t", [128, 8], BF16)
    bettd = din("bett", [128, 8], BF16)
    bnttd = din("bntt", [128, 8], BF16)
    hindd = din("hind", [128, 512], BF16)
    ht8d = din("ht8", [128, KC, FPC * H], F8)
    h_rmd = din("h_rm", [FPC * H, D], F32)
    pmatd = din("pmat", [FPC * H, FPC], BF16)
    ghid = din("ghi", [12, 128, KC, 512], F8)
    ghhd = din("ghh", [12, 128, KC, 512], F8)
    ghibd = din("ghib", [1, 3 * D], BF16)
    ghhbd = din("ghhb", [1, 3 * D], BF16)
    gsid = din("gsi", [12, 128, KC, 512], F8E3)
    gshd = din("gsh", [12, 128, KC, 512], F8E3)
    gsibd = din("gsib", [1, 3 * D], BF16)
    gshbd = din("gshb", [1, 3 * D], BF16)
    scsfd = din("scsf", [128, KC, 2 * FPC], BF16)
    sc4rmd = din("sc4rm", [FPC, D], F32)
    sfrmd = din("sfrm", [FPC, D], F32)
    outp = nc.dram_tensor("outp", [FPC, D], F32, kind="ExternalOutput")

    from contextlib import ExitStack

    with tile.TileContext(nc) as tc, ExitStack() as ctx:
        glob = ctx.enter_context(tc.tile_pool(name="glob", bufs=1))
        pbias = ctx.enter_context(tc.tile_pool(name="pbias", bufs=3, side="right"))

        oi_t = glob.tile([16, 544], BF16)
        ident16 = oi_t[0:16, 0:16]
        make_identity(nc, ident16)
        ones_b = oi_t[0:1, 32:544]
        nc.vector.memset(ones_b, 1.0)
        wb_t = glob.tile([128, 544], BF16)
        wl2_sb = wb_t[:, 0:8]
        nc.sync.dma_start(out=wl2_sb, in_=wl2.ap())
        bl1t_sb = wb_t[:, 8:16]
        nc.sync.dma_start(out=bl1t_sb, in_=bl1td.ap())
        bett_sb = wb_t[:, 16:24]
        nc.sync.dma_start(out=bett_sb, in_=bettd.ap())
        bntt_sb = wb_t[:, 24:32]
        nc.sync.dma_start(out=bntt_sb, in_=bnttd.ap())
        hind_sb = wb_t[:, 32:544]
        nc.sync.dma_start(out=hind_sb, in_=hindd.ap())

        f8pair = glob.tile([128, KC, 2 * FPC * H], F8)
        msum_f8 = f8pair[:, :, 0:FPC * H]
        ht8_sb = f8pair[:, :, FPC * H:2 * FPC * H]
        nc.scalar.dma_start(out=ht8_sb, in_=ht8d.ap())
        bfpack = glob.tile([128, KC, 4 * FPC], BF16)
        scsf_sb = bfpack[:, :, 0:2 * FPC]
        nc.scalar.dma_start(out=scsf_sb, in_=scsfd.ap())
        ah_sb = bfpack[:, :, 2 * FPC:3 * FPC]
        s1t_sb = bfpack[:, :, 3 * FPC:4 * FPC]

        bw_tiles = {}

        with tc.tile_pool(name="bw", bufs=2, side="right") as bwpool, \
             tc.tile_pool(name="pghp", bufs=1) as pghp:

            ghp_sb = pghp.tile([FPC * H, 12, 512], BF16)   # whh @ H + bhh, descaled

            def bw_load(j):
                wt = bwpool.tile([128, KC, 512], F8, tag="bw")
                nc.sync.dma_start(out=wt, in_=ghid.ap()[j])
                bw_tiles[j] = wt

            # ================= Phase A =================
            with (
                tc.tile_pool(name="pal", bufs=1) as pal,
                tc.tile_pool(name="pwcat", bufs=1) as pwcat,
                tc.tile_pool(name="pa", bufs=2) as pa,
                tc.tile_pool(name="pam", bufs=2) as pam,
                tc.tile_pool(name="prelu", bufs=1) as prelu,
                tc.tile_pool(name="pa1", bufs=1) as pa1,
                tc.tile_pool(name="pav", bufs=3) as pav,
                tc.tile_pool(name="pghw", bufs=2, side="right") as pghw,
            ):
                mn_f8 = pal.tile([128, 8, FPC * O], F8)       # mn^T, unscaled
                q8rm = pal.tile([128, 2, D // 2], BF16)       # 8 * (Wl1R mn), row-major
                xu_f = pal.tile([128, KC, FPC * H], F32)      # (me0u ⊕ mnu)^T
                xu_b = pal.tile([128, KC, FPC * H], BF16)
                msum_f = pal.tile([128, KC, FPC * H], F32)    # msum^T (raw sum over o)

                wcat_sb = pwcat.tile([128, KC, D], F8)
                nc.sync.dma_start(out=wcat_sb[:, :, 0:D // 2], in_=wcatA.ap())
                nc.sync.dma_start(out=wcat_sb[:, :, D // 2:D], in_=wcatB.ap())
                wl1l_sb = pwcat.tile([128, 8, D // 2], F8)
                nc.sync.dma_start(out=wl1l_sb, in_=wl1l.ap())

                # ---- Phase 0: mn^T = Wn O^T + bn; Q row-major ----
                with (
                    tc.tile_pool(name="p0", bufs=1) as p0,
                    tc.tile_pool(name="p0ps", bufs=4, space="PSUM") as p0ps,
                ):
                    wnt_sb = p0.tile([128, KC, D // 2], F8)
                    nc.scalar.dma_start(out=wnt_sb, in_=wnt.ap())
                    ot_sb = p0.tile([128, KC, FPC * O], F8)
                    nc.scalar.dma_start(out=ot_sb, in_=ot.ap())
                    wl1r_sb = p0.tile([128, 8, D // 2], F8)
                    nc.scalar.dma_start(out=wl1r_sb, in_=wl1r.ap())
                    for mt in range(8):
                        pm = p0ps.tile([128, FPC * O], F32, tag="pm")
                        for i in range(8):
                            nc.tensor.matmul(pm, lhsT=wnt_sb[:, 2 * i:2 * i + 2, mt * 128:(mt + 1) * 128],
                                             rhs=ot_sb[:, 2 * i:2 * i + 2, :],
                                             perf_mode=DR, start=(i == 0), stop=(i == 7))
                        nc.scalar.activation(mn_f8[:, mt, :], pm, AF.Identity,
                                             bias=bntt_sb[:, mt:mt + 1], scale=1.0 / WS)
                    # Q row-major: q8rm[fo-chunk c] = 8 * (mn @ Wl1R.T)
                    for c in range(2):
                        for n in range(2):
                            pq = p0ps.tile([128, 512], F32, tag="pq")
                            for i in range(4):
                                nc.tensor.matmul(pq, lhsT=mn_f8[:, 2 * i:2 * i + 2, c * 128:(c + 1) * 128],
                                                 rhs=wl1r_sb[:, 2 * i:2 * i + 2, n * 512:(n + 1) * 512],
                                                 perf_mode=DR, start=(i == 0), stop=(i == 3))
                            nc.scalar.copy(q8rm[:, c, n * 512:(n + 1) * 512], pq)

                with tc.tile_pool(name="paps", bufs=4, space="PSUM") as paps, \
                     tc.tile_pool(name="papss", bufs=1, space="PSUM") as papss, \
                     tc.tile_pool(name="papw", bufs=2, space="PSUM") as papw:

                    def softmax_block(relu_t, wtag):
                        pl = papss.tile([1, 512], F32, tag="pl")
                        for kc2 in range(8):
                            nc.tensor.matmul(pl, lhsT=wl2_sb[:, kc2:kc2 + 1],
                                             rhs=relu_t[:, kc2, :], start=(kc2 == 0), stop=(kc2 == 7))
                        pl3 = pl.rearrange("o (g i) -> o g i", i=16)
                        smx = pa1.tile([1, 640], F32, tag="smx")
                        mx, sm, rs = smx[:, 512:544], smx[:, 544:576], smx[:, 576:608]
                        sub = smx[:, 0:512]
                        nc.vector.reduce_max(mx, pl3, axis=AX.X)
                        nc.vector.tensor_tensor(sub.rearrange("o (g i) -> o g i", i=16), pl3,
                                                mx.broadcast_to((1, 32, 16)), op=ALU.subtract)
                        nc.scalar.activation(sub, sub, AF.Exp)
                        ex3 = sub.rearrange("o (g i) -> o g i", i=16)
                        nc.vector.reduce_sum(sm, ex3, axis=AX.X)
                        nc.vector.reciprocal(rs, sm)
                        w_sb = wbb[0:1, 3, :]
                        nc.vector.tensor_tensor(w_sb.rearrange("o (g i) -> o g i", i=16), ex3,
                                                rs.broadcast_to((1, 32, 16)), op=ALU.mult)
                        return w_sb

                    def broadcast_w(w_sb, bidx):
                        pw = papw.tile([128, 512], F32, tag="pw")
                        nc.tensor.matmul(pw, lhsT=ones_b[0:1, 0:128], rhs=w_sb,
                                         start=True, stop=True)
                        wb = wbb[:, bidx, :]
                        nc.scalar.copy(wb, pw)
                        return wb

                    for q in range(NQ):
                        xq = pa.tile([128, KC, 512], F8, tag="xq")
                        nc.sync.dma_start(out=xq, in_=e0t.ap()[q])
                        me0t = pam.tile([128, 8, 512], F8, tag="me0t")
                        relu_sb = prelu.tile([128, 8, 512], BF16, tag="relu")
                        wbb = pa1.tile([128, 4, 512], BF16, tag="wbb")

                        # step0: me0 = We E + be
                        for mt in range(8):
                            pe = paps.tile([128, 512], F32, tag="wave")
                            for i in range(8):
                                nc.tensor.matmul(pe, lhsT=wcat_sb[:, 2 * i:2 * i + 2, mt * 128:(mt + 1) * 128],
                                                 rhs=xq[:, 2 * i:2 * i + 2, :],
                                                 perf_mode=DR, start=(i == 0), stop=(i == 7))
                            nc.scalar.activation(me0t[:, mt, :], pe, AF.Identity,
                                                 bias=bett_sb[:, mt:mt + 1], scale=1.0 / WS)
                        # step0: a0 = relu(Wl1 E + bl1)
                        for mt in range(8, 16):
                            pe = paps.tile([128, 512], F32, tag="wave")
                            for i in range(8):
                                nc.tensor.matmul(pe, lhsT=wcat_sb[:, 2 * i:2 * i + 2, mt * 128:(mt + 1) * 128],
                                                 rhs=xq[:, 2 * i:2 * i + 2, :],
                                                 perf_mode=DR, start=(i == 0), stop=(i == 7))
                            nc.scalar.activation(relu_sb[:, mt - 8, :], pe, AF.Relu,
                                                 bias=bl1t_sb[:, mt - 8:mt - 7], scale=1.0 / WS)
                        w0_sb = softmax_block(relu_sb, "w0")
                        w0b = broadcast_w(w0_sb, 0)

                        # step1: a1 = relu(w0*(P+Q) + bl1), P = Wl1L me0
                        qbase = (q % 2) * 64
                        for mt in range(8):
                            pp = paps.tile([128, 512], F32, tag="wave")
                            for i in range(4):
                                nc.tensor.matmul(pp, lhsT=wl1l_sb[:, 2 * i:2 * i + 2, mt * 128:(mt + 1) * 128],
                                                 rhs=me0t[:, 2 * i:2 * i + 2, :],
                                                 perf_mode=DR, start=(i == 0), stop=False)
                            nc.tensor.matmul(pp, lhsT=q8rm[qbase:qbase + 64, q // 2, mt * 128:(mt + 1) * 128],
                                             rhs=hind_sb[qbase:qbase + 64, :],
                                             start=False, stop=True)
                            v2 = pav.tile([128, 512], BF16, tag="v")
                            nc.vector.tensor_tensor(v2, pp, w0b, op=ALU.mult)
                            nc.scalar.activation(relu_sb[:, mt, :], v2, AF.Relu,
                                                 bias=bl1t_sb[:, mt:mt + 1], scale=1.0 / WS)
                        w1_sb = softmax_block(relu_sb, "w1")
                        w1b = broadcast_w(w1_sb, 1)
                        ub = wbb[:, 2, :]
                        nc.vector.tensor_tensor(ub, w0b, w1b, op=ALU.mult)

                        # weighted reductions over o
                        for kc in range(8):
                            tmp = pav.tile([128, 512], BF16, tag="v")
                            nc.vector.tensor_tensor(tmp, me0t[:, kc, :], ub, op=ALU.mult)
                            nc.vector.reduce_sum(xu_f[:, kc, q * 32:(q + 1) * 32], _r4(tmp), axis=AX.X)
                        for kc in range(8):
                            tmp = pav.tile([128, 512], BF16, tag="v")
                            nc.vector.tensor_tensor(_r4(tmp), _bc4(mn_f8, kc, q), _r4(ub), op=ALU.mult)
                            nc.vector.reduce_sum(xu_f[:, 8 + kc, q * 32:(q + 1) * 32], _r4(tmp), axis=AX.X)
                            tmp2 = pav.tile([128, 512], BF16, tag="v")
                            nc.vector.tensor_tensor(_r4(tmp2), _bc4(mn_f8, kc, q), _r4(w1b), op=ALU.mult)
                            nc.vector.reduce_sum(msum_f[:, 8 + kc, q * 32:(q + 1) * 32], _r4(tmp2), axis=AX.X)

                        # whh-gates for 3 human-GRU blocks (weights consumed now)
                        for j in range(3 * q, 3 * q + 3):
                            wt = pghw.tile([128, KC, 512], F8, tag="ghw")
                            nc.scalar.dma_start(out=wt, in_=ghhd.ap()[j])
                            pg = papw.tile([128, 512], F32, tag="pw")
                            for i in range(8):
                                nc.tensor.matmul(pg, lhsT=ht8_sb[:, 2 * i:2 * i + 2, :],
                                                 rhs=wt[:, 2 * i:2 * i + 2, :],
                                                 perf_mode=DR, start=(i == 0), stop=False)
                            bb = pbias.tile([1, 512], BF16, tag="bias")
                            nc.sync.dma_start(out=bb, in_=ghhbd.ap()[:, j * 512:(j + 1) * 512])
                            nc.tensor.matmul(pg, lhsT=ones_b[0:1, 0:FPC * H], rhs=bb,
                                             start=False, stop=True)
                            nc.scalar.activation(ghp_sb[:, j, :], pg, AF.Copy, scale=1.0 / WS)

                    for kc in range(KC):
                        nc.vector.tensor_copy(xu_b[:, kc, :], xu_f[:, kc, :])
                    # folded msum_e = We (me0u ⊕ mnu) + be
                    for mt in range(8):
                        pf = papw.tile([128, FPC * H], F32, tag="pw")
                        for kc in range(KC):
                            nc.tensor.matmul(pf, lhsT=wcat_sb[:, kc, mt * 128:(mt + 1) * 128],
                                             rhs=xu_b[:, kc, :], start=(kc == 0), stop=(kc == KC - 1))
                        nc.scalar.activation(msum_f[:, mt, :], pf, AF.Identity,
                                             bias=bett_sb[:, mt:mt + 1], scale=1.0 / WS)
                    for kc in range(KC):
                        nc.vector.tensor_copy(msum_f8[:, kc, :], msum_f[:, kc, :])
                    bw_load(0)  # prefetch first ghi block

            # ============ Phase B (with PH pass interleaved) ============
            with tc.tile_pool(name="pcg1", bufs=1) as pcg1:
                gh1_sb = pcg1.tile([FPC, 12, 512], BF16)   # whh Sc4 + bhh (descaled)
                gh2_sb = pcg1.tile([FPC, 12, 512], BF16)   # whh Sf + bhh
                with (
                    tc.tile_pool(name="pcw", bufs=2, side="right") as pcw,
                    tc.tile_pool(name="pchps", bufs=1, space="PSUM") as pchps,
                    tc.tile_pool(name="pbh", bufs=1) as pbh,
                    tc.tile_pool(name="pbt", bufs=2) as pbt,
                    tc.tile_pool(name="pbps", bufs=1, space="PSUM") as pbps,
                    tc.tile_pool(name="pbps2", bufs=2, space="PSUM") as pbps2,
                ):
                    NR = FPC * H  # 128 rows
                    h_rm = pbh.tile([NR, D], F32)
                    nc.sync.dma_start(out=h_rm, in_=h_rmd.ap())
                    pmat_sb = pbh.tile([NR, FPC], BF16)
                    nc.sync.dma_start(out=pmat_sb, in_=pmatd.ap())
                    hum_b = pbh.tile([NR, D], BF16)

                    def ph_pack(jp):
                        """gh1/gh2 = whh [Sc4|Sf] + bhh for j-blocks jp*2, jp*2+1."""
                        pch1 = pchps.tile([128, 512], F32, tag="pch1")
                        pch2 = pchps.tile([128, 512], F32, tag="pch2")
                        wts = []
                        for g in range(2):
                            j = jp * 2 + g
                            wt = pcw.tile([128, KC, 512], F8E3, tag="cw")
                            nc.scalar.dma_start(out=wt, in_=gshd.ap()[j])
                            wts.append(wt)
                        for kc in range(KC):
                            for g in range(2):
                                nc.tensor.matmul(pch1[32 * g:32 * g + 16, :],
                                                 lhsT=scsf_sb[:, kc, 0:FPC], rhs=wts[g][:, kc, :],
                                                 tile_position=(0, 32 * g),
                                                 start=(kc == 0), stop=False, skip_group_check=True)
                                nc.tensor.matmul(pch2[32 * g:32 * g + 16, :],
                                                 lhsT=scsf_sb[:, kc, FPC:2 * FPC], rhs=wts[g][:, kc, :],
                                                 tile_position=(0, 32 * g),
                                                 start=(kc == 0), stop=False, skip_group_check=True)
                        for g in range(2):
                            j = jp * 2 + g
                            bsh = pbias.tile([1, 512], BF16, tag="bias")
                            nc.sync.dma_start(out=bsh, in_=gshbd.ap()[:, j * 512:(j + 1) * 512])
                            nc.tensor.matmul(pch1[32 * g:32 * g + 16, :], lhsT=ones_b[0:1, 0:16],
                                             rhs=bsh, tile_position=(0, 32 * g),
                                             start=False, stop=True, skip_group_check=True)
                            nc.tensor.matmul(pch2[32 * g:32 * g + 16, :], lhsT=ones_b[0:1, 0:16],
                                             rhs=bsh, tile_position=(0, 32 * g),
                                             start=False, stop=True, skip_group_check=True)
                        for g in range(2):
                            j = jp * 2 + g
                            nc.scalar.activation(gh1_sb[:, j, :], pch1[32 * g:32 * g + 16, :],
                                                 AF.Copy, scale=1.0 / WS3)
                            nc.scalar.activation(gh2_sb[:, j, :], pch2[32 * g:32 * g + 16, :],
                                                 AF.Copy, scale=1.0 / WS3)

                    def gi_block(j, pt):
                        """gi-half: (wih/O) msum + bih into psum (x8)."""
                        if j not in bw_tiles:
                            bw_load(j)
                        wt = bw_tiles[j]
                        for i in range(8):
                            nc.tensor.matmul(pt, lhsT=msum_f8[:, 2 * i:2 * i + 2, :],
                                             rhs=wt[:, 2 * i:2 * i + 2, :],
                                             perf_mode=DR, start=(i == 0), stop=False)
                        bb = pbias.tile([1, 512], BF16, tag="bias")
                        nc.sync.dma_start(out=bb, in_=ghibd.ap()[:, j * 512:(j + 1) * 512])
                        nc.tensor.matmul(pt, lhsT=ones_b[0:1, 0:NR], rhs=bb,
                                         start=False, stop=True)

                    for t in range(4):
                        cols = slice(t * 512, (t + 1) * 512)
                        p_r = pbps.tile([NR, 512], F32, tag="pr")
                        gi_block(t, p_r)
                        p_z = pbps.tile([NR, 512], F32, tag="pz")
                        gi_block(4 + t, p_z)
                        p_in = pbps.tile([NR, 512], F32, tag="pin")
                        gi_block(8 + t, p_in)
                        pre_r = pbt.tile([NR, 512], F32, tag="tt")
                        nc.vector.scalar_tensor_tensor(pre_r, p_r, 1.0 / WS, ghp_sb[:, t, :],
                                                       op0=ALU.mult, op1=ALU.add)
                        r_sb = pbh.tile([NR, 512], F32, tag="r")
                        nc.scalar.activation(r_sb, pre_r, AF.Sigmoid)
                        pre_z = pbt.tile([NR, 512], F32, tag="tt")
                        nc.vector.scalar_tensor_tensor(pre_z, p_z, 1.0 / WS, ghp_sb[:, 4 + t, :],
                                                       op0=ALU.mult, op1=ALU.add)
                        z_sb = pbh.tile([NR, 512], F32, tag="z")
                        nc.scalar.activation(z_sb, pre_z, AF.Sigmoid)
                        t1 = pbt.tile([NR, 512], F32, tag="tt")
                        nc.vector.tensor_tensor(t1, r_sb, ghp_sb[:, 8 + t, :], op=ALU.mult)
                        t2 = pbt.tile([NR, 512], F32, tag="tt")
                        nc.vector.scalar_tensor_tensor(t2, p_in, 1.0 / WS, t1,
                                                       op0=ALU.mult, op1=ALU.add)
                        n_sb = pbh.tile([NR, 512], F32, tag="n")
                        nc.scalar.activation(n_sb, t2, AF.Tanh)
                        t3 = pbt.tile([NR, 512], F32, tag="tt")
                        nc.vector.tensor_tensor(t3, h_rm[:, cols], n_sb, op=ALU.subtract)
                        t4 = pbt.tile([NR, 512], F32, tag="tt")
                        nc.vector.tensor_tensor(t4, z_sb, t3, op=ALU.mult)
                        nc.vector.tensor_tensor(hum_b[:, cols], n_sb, t4, op=ALU.add)
                        if t < 3:
                            ph_pack(2 * t)
                            ph_pack(2 * t + 1)
                    for c in range(KC):
                        pah = pbps2.tile([128, FPC], F32, tag="pah")
                        nc.tensor.matmul(pah, lhsT=hum_b[:, c * 128:(c + 1) * 128], rhs=pmat_sb,
                                         start=True, stop=True)
                        nc.scalar.copy(ah_sb[:, c, :], pah)

                # ============ Phase C: two S-node GRUs ============
                with (
                    tc.tile_pool(name="pc1", bufs=1) as pc1,
                    tc.tile_pool(name="pct", bufs=2) as pct,
                    tc.tile_pool(name="pcgi", bufs=1, side="right") as pcgi,
                    tc.tile_pool(name="pci", bufs=3, side="right") as pci,
                    tc.tile_pool(name="pcps", bufs=2, space="PSUM") as pcps,
                    tc.tile_pool(name="pctps", bufs=2, space="PSUM") as pctps,
                ):
                    sc4rm_sb = pc1.tile([FPC, D], F32)
                    nc.sync.dma_start(out=sc4rm_sb, in_=sc4rmd.ap())
                    sfrm_sb = pc1.tile([FPC, D], F32)
                    nc.sync.dma_start(out=sfrm_sb, in_=sfrmd.ap())
                    g_sb = pc1.tile([FPC, 8, 512], BF16, tag="g")     # r,z gates
                    gn_sb = pc1.tile([FPC, 4, 512], BF16, tag="gn")   # inn
                    s1_sb = pc1.tile([FPC, D], BF16)
                    out32 = pc1.tile([FPC, D], F32)

                    gsi_keep = {}

                    def gsi_tile(j):
                        if j < 6:
                            if j not in gsi_keep:
                                wt = pcgi.tile([128, KC, 512], F8E3, tag=f"gk{j}")
                                nc.scalar.dma_start(out=wt, in_=gsid.ap()[j])
                                gsi_keep[j] = wt
                            return gsi_keep[j]
                        wt = pci.tile([128, KC, 512], F8E3, tag="ci")
                        nc.scalar.dma_start(out=wt, in_=gsid.ap()[j])
                        return wt

                    def gi_pass(xt, gh_src):
                        """gi = wih x + bih (x64); g = gi/64 + gh for r,z; gi/64 for n."""
                        for jp in range(6):
                            pci_ps = pcps.tile([128, 512], F32, tag="pch")
                            wts = [gsi_tile(jp * 2), gsi_tile(jp * 2 + 1)]
                            for kc in range(KC):
                                for g in range(2):
                                    nc.tensor.matmul(pci_ps[32 * g:32 * g + 16, :],
                                                     lhsT=xt[:, kc, :], rhs=wts[g][:, kc, :],
                                                     tile_position=(0, 32 * g),
                                                     start=(kc == 0), stop=False, skip_group_check=True)
                            for g in range(2):
                                j = jp * 2 + g
                                bsi = pbias.tile([1, 512], BF16, tag="bias")
                                nc.sync.dma_start(out=bsi, in_=gsibd.ap()[:, j * 512:(j + 1) * 512])
                                nc.tensor.matmul(pci_ps[32 * g:32 * g + 16, :], lhsT=ones_b[0:1, 0:16],
                                                 rhs=bsi, tile_position=(0, 32 * g),
                                                 start=False, stop=True, skip_group_check=True)
                            for g in range(2):
                                j = jp * 2 + g
                                if j < 8:
                                    nc.vector.scalar_tensor_tensor(
                                        g_sb[:, j, :], pci_ps[32 * g:32 * g + 16, :], 1.0 / WS3,
                                        gh_src[:, j, :], op0=ALU.mult, op1=ALU.add)
                                else:
                                    nc.scalar.activation(gn_sb[:, j - 8, :], pci_ps[32 * g:32 * g + 16, :],
                                                         AF.Copy, scale=1.0 / WS3)

                    def s_elementwise(gh_src, hprev, outt):
                        for t in range(4):
                            cols = slice(t * 512, (t + 1) * 512)
                            r1 = pc1.tile([FPC, 512], F32, tag="c_r")
                            nc.scalar.activation(r1, g_sb[:, t, :], AF.Sigmoid)
                            z1 = pc1.tile([FPC, 512], F32, tag="c_z")
                            nc.scalar.activation(z1, g_sb[:, 4 + t, :], AF.Sigmoid)
                            u1 = pct.tile([FPC, 512], F32, tag="cu")
                            nc.vector.tensor_tensor(u1, r1, gh_src[:, 8 + t, :], op=ALU.mult)
                            u2 = pct.tile([FPC, 512], F32, tag="cu")
                            nc.vector.tensor_tensor(u2, u1, gn_sb[:, t, :], op=ALU.add)
                            n1 = pc1.tile([FPC, 512], F32, tag="c_n")
                            nc.scalar.activation(n1, u2, AF.Tanh)
                            u3 = pct.tile([FPC, 512], F32, tag="cu")
                            nc.vector.tensor_tensor(u3, hprev[:, cols], n1, op=ALU.subtract)
                            u4 = pct.tile([FPC, 512], F32, tag="cu")
                            nc.vector.tensor_tensor(u4, z1, u3, op=ALU.mult)
                            nc.vector.tensor_tensor(outt[:, cols], n1, u4, op=ALU.add)

                    gi_pass(ah_sb, gh1_sb)
                    s_elementwise(gh1_sb, sc4rm_sb, s1_sb)
                    for c in range(KC):
                        ptp = pctps.tile([128, 16], BF16, tag="tp")
                        nc.tensor.transpose(ptp, s1_sb[:, c * 128:(c + 1) * 128], ident16)
                        nc.scalar.copy(s1t_sb[:, c, :], ptp)
                    gi_pass(s1t_sb, gh2_sb)
                    s_elementwise(gh2_sb, sfrm_sb, out32)
                    nc.sync.dma_start(out=outp.ap(), in_=out32)

    nc.compile()
    return nc


def _tile_w(WT, blocks):
    """[2048, blocks*512] -> [blocks, 128, 16, 512] (pre-tiled for contiguous DMA)."""
    return np.ascontiguousarray(
        WT.reshape(16, 128, blocks, 512).transpose(2, 1, 0, 3))


def _tile_k(WT):
    """[2048, N] -> [128, 16, N]."""
    n = WT.shape[1]
    return np.ascontiguousarray(WT.reshape(16, 128, n).transpose(1, 0, 2))


def _make_hind():
    """h-broadcast indicator: hind[p, f*128+h*16+o] = (p%64 == f*16+o)."""
    m = np.zeros((128, 512), dtype=NB)
    for n in range(512):
        f, o = n // 128, n % 16
        m[f * 16 + o, n] = 1.0
        m[64 + f * 16 + o, n] = 1.0
    return m


def _prep_in_maps(inputs):
    E = np.ascontiguousarray(inputs["H_O_edges"].reshape(NFRAMES, ROWS, D))
    On = inputs["O_nodes"].reshape(NFRAMES, O, D)
    Hn = inputs["H_nodes"].reshape(NFRAMES, H, D)
    Sc4 = inputs["S_node_C4"].reshape(NFRAMES, D)
    Sf = np.ascontiguousarray(inputs["final_S_node"].transpose(0, 2, 1)).reshape(NFRAMES, D)

    We, Wl1, Wn = inputs["We"], inputs["Wl1"], inputs["Wn"]

    shared = {
        "wcatA": _tile_k((We * WS).T.astype(NE4)),
        "wcatB": _tile_k((Wl1 * WS).T.astype(NE4)),
        "wl1l": np.ascontiguousarray(
            (Wl1[:, :D // 2] * WS).T.astype(NE4).reshape(8, 128, D // 2).transpose(1, 0, 2)),
        "wl1r": np.ascontiguousarray(
            (Wl1[:, D // 2:] * WS).T.astype(NE4).reshape(8, 128, D // 2).transpose(1, 0, 2)),
        "wnt": _tile_k((Wn * WS).T.astype(NE4)),
        "wl2": np.ascontiguousarray(inputs["Wl2"][0].reshape(8, 128).T).astype(NB),
        "bl1t": np.ascontiguousarray(inputs["bl1"].reshape(8, 128).T).astype(NB),
        "bett": np.ascontiguousarray(inputs["be"].reshape(8, 128).T).astype(NB),
        "bntt": np.ascontiguousarray(inputs["bn"].reshape(8, 128).T).astype(NB),
        "hind": _make_hind(),
        "pmat": np.ascontiguousarray(np.kron(np.eye(FPC), np.ones((H, 1))) / H).astype(NB),
        "ghi": _tile_w((inputs["gh_wih"] * (WS / O)).T.astype(NE4), 12),
        "ghh": _tile_w((inputs["gh_whh"] * WS).T.astype(NE4), 12),
        "ghib": (inputs["gh_bih"] * WS)[None, :].astype(NB),
        "ghhb": (inputs["gh_bhh"] * WS)[None, :].astype(NB),
        "gsi": _tile_w((inputs["gs_wih"] * WS3).T.astype(NE3), 12),
        "gsh": _tile_w((inputs["gs_whh"] * WS3).T.astype(NE3), 12),
        "gsib": (inputs["gs_bih"] * WS3)[None, :].astype(NB),
        "gshb": (inputs["gs_bhh"] * WS3)[None, :].astype(NB),
    }

    in_maps = []
    for c in range(NCORES):
        fr = slice(c * FPC, (c + 1) * FPC)
        Ec = E[fr]  # [16, 128, 2048]
        e0t = np.ascontiguousarray(
            Ec.reshape(NQ, 4, ROWS, D).transpose(0, 3, 1, 2)
            .reshape(NQ, 16, 128, 512).transpose(0, 2, 1, 3)).astype(NE4)
        m = dict(shared)
        m.update({
            "e0t": e0t,
            "ot": _tile_k(On[fr].reshape(FPC * O, D).T.astype(NE4)),
            "ht8": _tile_k(Hn[fr].reshape(FPC * H, D).T.astype(NE4)),
            "h_rm": np.ascontiguousarray(Hn[fr].reshape(FPC * H, D)).astype(np.float32),
            "scsf": _tile_k(np.concatenate([Sc4[fr].T, Sf[fr].T], axis=1).astype(NB)),
            "sc4rm": np.ascontiguousarray(Sc4[fr]).astype(np.float32),
            "sfrm": np.ascontiguousarray(Sf[fr]).astype(np.float32),
        })
        in_maps.append(m)
    return in_maps


LAST_RESULT = None


def kernel(**inputs):
    global LAST_RESULT
    if "nc" not in _CACHE:
        _CACHE["nc"] = _build_nc()
    nc = _CACHE["nc"]
    in_maps = _prep_in_maps(inputs)
    trace = os.environ.get("KERNEL_TRACE", "0") == "1"
    res = bass_utils.run_bass_kernel_spmd(
        nc, in_maps, core_ids=list(range(NCORES)), trace=trace)
    LAST_RESULT = res
    out = np.concatenate([res.results[c]["outp"] for c in range(NCORES)], axis=0)
    return np.ascontiguousarray(out.reshape(B, F, D)).astype(np.float32)
